# revision 18
# baseline (speedup 1.0000x reference)
"""Causal single-head attention on 8 TRN2 NeuronCores.

Problem: x[B=8,T=2048,E=1024] fp32, per-head Q/K/V projections (D=128) +
causal softmax attention. Sharding: data-parallel over batch B — one batch
element per core; Q/K/V weights replicated.

Key structure (v3):
  - All inputs host-packed to the exact SBUF layout so every DMA is a
    contiguous [128, L] transfer: x as [128, (tile, chunk, 512)] in fp16
    and fp8, weights as [128, (chunk, 128)].
  - Q/K projections in fp8 e4m3 + DoubleRow (contraction 256/matmul) for
    t-tiles 1-3; t-tile 0 runs them in fp16 because its rows have few
    softmax terms, so q/k quantization noise doesn't average out there
    (it also produces k for s<512 in fp16, which every later tile reuses).
    V is always fp16: v errors hit the output directly.
    Wq/Wk/bq/bk pre-scaled by 32 on host (else ~50% of W mass is
    e4m3-subnormal); exp() scale absorbs 1/32^2.
  - Short PE warmup on the identity tile spans the initial DMA wait so
    HAM reaches 8/8 before the real matmuls.
  - Scores transposed S^T [s,128, t 512] = kT_slice @ qT, exp on ACT,
    diagonal masked by one 128x128 lower-triangle multiply; s-chunks
    above the diagonal skipped.
  - PV natural per 128-row t-chunk with ones-augmented v (denominator
    rides in psum column 128); PV runs one t-tile behind the scores so
    the exp chain never stalls the PE — except the last tile, whose PV
    chunks interleave into the scores stream to shorten the serial tail.
"""

import numpy as np

B, T, E, D = 8, 2048, 1024, 128
NT = 512                 # t-tile width (PSUM bank = 512 fp32)
N_TT = T // NT           # 4 t-tiles
N_TC = NT // 128         # 4 t-chunks per t-tile
N_EC = E // 128          # 8 e-chunks
N_SC = T // 128          # 16 s-chunks
VS = 132                 # v_aug free stride (129 used)
USE_FP8 = True
WSCALE = 32.0 if USE_FP8 else 1.0  # host pre-scale on Wq/Wk/bq/bk
SCALE = float(1.0 / (WSCALE * WSCALE * np.sqrt(D)))
N_WARM = 8

_cache: dict = {}


def _build(causal: bool):
    from contextlib import ExitStack
    import concourse.bass as bass
    import concourse.tile as tile
    from concourse import bacc, mybir
    from concourse.masks import make_identity

    f32 = mybir.dt.float32
    f16 = mybir.dt.float16
    f8 = mybir.dt.float8e4
    AF = mybir.ActivationFunctionType
    DR = mybir.MatmulPerfMode.DoubleRow

    nc = bacc.Bacc("TRN2", target_bir_lowering=False, debug=False,
                   num_devices=B)
    # packed inputs: [128, (tile, chunk, NT)] with x[t, e] at
    # [e % 128, tile(t), chunk(e), t % NT]
    xP16 = nc.dram_tensor("xP16", (128, N_TT * N_EC * NT), f16,
                          kind="ExternalInput").ap()
    Wq16 = nc.dram_tensor("Wq16", (128, N_EC * D), f16, kind="ExternalInput").ap()
    Wk16 = nc.dram_tensor("Wk16", (128, N_EC * D), f16, kind="ExternalInput").ap()
    Wv16 = nc.dram_tensor("Wv16", (128, N_EC * D), f16, kind="ExternalInput").ap()
    xP8 = Wq8 = Wk8 = None
    if USE_FP8:
        xP8 = nc.dram_tensor("xP8", (128, N_EC * NT), f8,
                             kind="ExternalInput").ap()
        Wq8 = nc.dram_tensor("Wq8", (128, N_EC * D), f8, kind="ExternalInput").ap()
        Wk8 = nc.dram_tensor("Wk8", (128, N_EC * D), f8, kind="ExternalInput").ap()
    bs = {p: nc.dram_tensor(f"b{p}", (D, 1), f32, kind="ExternalInput").ap()
          for p in "qkv"}
    out = nc.dram_tensor("out", (T, D), f16, kind="ExternalOutput").ap()

    with tile.TileContext(nc) as tc, ExitStack() as ctx:
        consts = ctx.enter_context(tc.tile_pool(name="consts", bufs=1))
        xt8_pool = ctx.enter_context(tc.tile_pool(name="xt8", bufs=3))
        xt16_pool = ctx.enter_context(tc.tile_pool(name="xt16", bufs=3))
        qT_pool = ctx.enter_context(tc.tile_pool(name="qT", bufs=2))
        vT_pool = ctx.enter_context(tc.tile_pool(name="vT", bufs=2))
        ex_pool = ctx.enter_context(tc.tile_pool(name="ex", bufs=3))
        small = ctx.enter_context(tc.tile_pool(name="small", bufs=8))
        outp = ctx.enter_context(tc.tile_pool(name="outp", bufs=8))
        ps_qkv = ctx.enter_context(tc.tile_pool(name="ps_qkv", bufs=2,
                                                space="PSUM"))
        ps_s = ctx.enter_context(tc.tile_pool(name="ps_s", bufs=3,
                                              space="PSUM"))
        ps_o = ctx.enter_context(tc.tile_pool(name="ps_o", bufs=2,
                                              space="PSUM"))
        ps_t = ctx.enter_context(tc.tile_pool(name="ps_t", bufs=1,
                                              space="PSUM"))

        # ---- DMAs: scalar ring carries weights, sync ring carries x ----
        # tile 0 computes q/k in fp16, so its critical path is Wq16+xt16_0.
        w16 = {p: consts.tile([128, N_EC, D], f16, tag=f"w16{p}",
                              name=f"w16{p}")
               for p in "qkv"}
        nc.scalar.dma_start(w16["q"][:].rearrange("p c d -> p (c d)"), Wq16[:])

        def load_x(pool, tag, dram, jj, dt, eng, pieces=1):
            # an x-tile as `pieces` separate tiles so dependencies unlock
            # per piece (tile deps are whole-tile, not subtile); each
            # piece is itself two chunk-range DMA transfers
            w = N_EC // pieces
            ph = []
            for h in range(pieces):
                xt = pool.tile([128, w, NT], dt, tag=f"{tag}{h}",
                               name=f"{tag}{h}")
                for g in range(2):
                    c0 = w * h + (w // 2) * g
                    t0 = (jj * N_EC + c0) * NT
                    eng.dma_start(
                        xt[:, (w // 2) * g:(w // 2) * (g + 1), :]
                        .rearrange("p c n -> p (c n)"),
                        dram[:, t0:t0 + (w // 2) * NT])
                ph.append((xt, w * h, w))
            return ph

        def xch(handle, c):
            # chunk accessor over a piece-list x handle
            for xt, c0, w in handle:
                if c0 <= c < c0 + w:
                    return xt[:, c - c0, :]
            raise KeyError(c)

        def xch2(handle, c):
            # two-chunk slice (DoubleRow) — never crosses a piece boundary
            for xt, c0, w in handle:
                if c0 <= 2 * c and 2 * c + 2 <= c0 + w:
                    return xt[:, 2 * c - c0:2 * c - c0 + 2, :]
            raise KeyError(c)

        def load_x16(jj):
            return load_x(xt16_pool, "xt16", xP16, jj, f16, nc.sync)

        def load_x8_t1():
            # only t-tile 1's fp8 x comes over DMA (tile 0 is fp16-only;
            # tiles 2-3 are cast from their fp16 tiles by the idle gpsimd
            # engine, saving 1.5MB of HBM input stream)
            return load_x(xt8_pool, "xt8", xP8, 0, f8, nc.sync)

        def cast_x8(handle):
            xt = xt8_pool.tile([128, N_EC, NT], f8, tag="xt80",
                               name="xt8c")
            (src16, _, _), = handle
            for h in range(2):
                nc.gpsimd.tensor_copy(xt[:, 4 * h:4 * h + 4, :],
                                      src16[:, 4 * h:4 * h + 4, :])
            return [(xt, 0, N_EC)]

        xt16_tiles = {0: load_x(xt16_pool, "xt16q", xP16, 0, f16, nc.sync,
                                pieces=4)}
        nc.scalar.dma_start(w16["k"][:].rearrange("p c d -> p (c d)"), Wk16[:])
        b_t = {}
        for p in "qkv":
            bt = consts.tile([128, 1], f32, tag=f"b{p}")
            nc.scalar.dma_start(bt[:], bs[p])
            b_t[p] = bt
        nc.scalar.dma_start(w16["v"][:].rearrange("p c d -> p (c d)"), Wv16[:])
        w8 = {}
        if USE_FP8:
            for p, W8 in (("q", Wq8), ("k", Wk8)):
                wt = consts.tile([128, N_EC, D], f8, tag=f"w8{p}")
                nc.scalar.dma_start(wt[:].rearrange("p c d -> p (c d)"), W8[:])
                w8[p] = wt
            xt8_tiles = {1: load_x8_t1()} if N_TT > 1 else {}
        else:
            xt8_tiles = {}
        if N_TT > 1:
            xt16_tiles[1] = load_x16(1)

        ident_h = consts.tile([128, 128], f16, tag="ident_h")
        make_identity(nc, ident_h[:])

        # PE warmup while the first x tile lands: HAM needs ~3.4us of
        # sustained PE activity to lift the clock gate to 8/8.  Routing
        # every warmup matmul through the single ps_t buffer serializes
        # them on the WAW hazard (~0.4us apiece), stretching a handful of
        # matmuls across the whole DMA wait.
        warm_w = consts.tile([128, NT], f16, tag="warm_w")
        nc.vector.memset(warm_w[:], 0.0)
        for _ in range(N_WARM):
            pw = ps_s.tile([128, NT], f32, tag="ps_s")
            nc.tensor.matmul(pw[:], ident_h[:], warm_w[:],
                             start=True, stop=True)

        masks_h = None
        if causal:
            # single lower-triangular (keep t>=s) 128x128 block
            masks_h = consts.tile([128, 128], f16, tag="masks_h")
            nc.gpsimd.memset(masks_h[:], 1.0)
            nc.gpsimd.affine_select(
                out=masks_h[:], in_=masks_h[:],
                compare_op=mybir.AluOpType.is_ge,
                fill=0.0, base=0, channel_multiplier=-1,
                pattern=[[1, 128]])

        kT_all = consts.tile([128, T], f16, tag="kT_all")
        v_all = consts.tile([128, N_SC, VS], f16, tag="v_all")
        nc.vector.memset(v_all[:, :, 128:129], 1.0)  # the ones columns

        qT_all = None
        if not causal:
            # full attention needs every t-tile's q resident before phase 2
            qT_all = consts.tile([128, T], f16, tag="qT_all")

        def proj(w16_t, w8_t, xt16, xt8, dest, bias, fp8):
            ps = ps_qkv.tile([128, NT], f32, tag="ps_qkv")
            if fp8:
                for c in range(N_EC // 2):
                    nc.tensor.matmul(
                        ps[:], w8_t[:, 2 * c:2 * c + 2, :], xch2(xt8, c),
                        start=(c == 0), stop=(c == N_EC // 2 - 1),
                        perf_mode=DR)
            else:
                for c in range(N_EC):
                    nc.tensor.matmul(
                        ps[:], w16_t[:, c, :], xch(xt16, c),
                        start=(c == 0), stop=(c == N_EC - 1))
            nc.vector.tensor_scalar_add(dest, ps[:], bias[:])

        def proj_qk(p, j, xt16, xt8, dest):
            fp8 = USE_FP8 and j > 0
            proj(w16[p], w8.get(p), xt16, xt8, dest, b_t[p], fp8)

        def proj_qk_fused16(xt16, qdest, kdest):
            # tile 0 only: q and k accumulate interleaved into two psum
            # banks so each arriving x piece feeds 4 back-to-back matmuls
            # (piece arrival paces the PE here; sequential q-then-k would
            # outrun the DMA on q and then replay k from sbuf cold)
            psq = ps_qkv.tile([128, NT], f32, tag="ps_qkv")
            psk = ps_qkv.tile([128, NT], f32, tag="ps_qkv")
            for c in range(N_EC):
                xc = xch(xt16, c)
                nc.tensor.matmul(psq[:], w16["q"][:, c, :], xc,
                                 start=(c == 0), stop=(c == N_EC - 1),
                                 skip_group_check=True)
                nc.tensor.matmul(psk[:], w16["k"][:, c, :], xc,
                                 start=(c == 0), stop=(c == N_EC - 1),
                                 skip_group_check=True)
            nc.vector.tensor_scalar_add(qdest, psq[:], b_t["q"][:])
            nc.vector.tensor_scalar_add(kdest, psk[:], b_t["k"][:])

        def scores_exp(j, qT, ex_all, i0, i1):
            # Diagonal s-chunk m: columns t_local < 128*m are never read by
            # PV (those t-chunks exclude this s-chunk), so compute only
            # [128*m:NT] and mask just the 128-wide diagonal sub-block.
            for i in range(i0, i1):
                m = i - j * N_TC
                off = 128 * m if (causal and m > 0) else 0
                ps = ps_s.tile([128, NT], f32, tag="ps_s")
                nc.tensor.matmul(ps[:, off:NT],
                                 kT_all[:, i * 128:(i + 1) * 128],
                                 qT[:, off:NT], start=True, stop=True)
                ex = ex_all[:, i * NT + off:(i + 1) * NT]
                nc.scalar.activation(ex, ps[:, off:NT], AF.Exp, scale=SCALE)
                if causal and m >= 0:
                    nc.vector.tensor_mul(
                        ex_all[:, i * NT + off:i * NT + off + 128],
                        ex_all[:, i * NT + off:i * NT + off + 128],
                        masks_h[:])

        def v_proj_transpose(j, xt16):
            vT = vT_pool.tile([128, NT], f16, tag="vT")
            proj(w16["v"], None, xt16, None, vT[:], b_t["v"], False)
            for tch in range(N_TC):
                sc = j * N_TC + tch
                pt = ps_t.tile([128, 256], f16, tag="ps_t")
                nc.tensor.transpose(pt[:, 0:128],
                                    vT[:, tch * 128:(tch + 1) * 128],
                                    ident_h[:])
                nc.vector.tensor_copy(v_all[:, sc, 0:128], pt[:, 0:128])

        def pv_chunk(j, ex_all, tch):
            # PV natural for one t-chunk; denominator rides in column 128
            tc_glob = j * N_TC + tch
            n_i = tc_glob + 1 if causal else N_SC
            po = ps_o.tile([128, VS], f32, tag="ps_o")
            for i in range(n_i):
                nc.tensor.matmul(
                    po[:, 0:129],
                    ex_all[:, i * NT + tch * 128:i * NT + (tch + 1) * 128],
                    v_all[:, i, 0:129],
                    start=(i == 0), stop=(i == n_i - 1),
                    skip_group_check=True)
            rec = small.tile([128, 1], f32, tag="rec")
            nc.vector.reciprocal(rec[:], po[:, 128:129])
            ot = outp.tile([128, 128], f16, tag="ot")
            nc.vector.tensor_scalar_mul(ot[:], po[:, 0:128], rec[:])
            r0 = tc_glob * 128
            nc.sync.dma_start(out[r0:r0 + 128, :], ot[:])

        def pv_out(j, ex_all):
            for tch in range(N_TC):
                pv_chunk(j, ex_all, tch)

        if causal:
            prev = None
            for j in range(N_TT):
                t0 = j * NT
                last = j == N_TT - 1
                xt16 = xt16_tiles.pop(j)
                xt8 = xt8_tiles.pop(j, None)
                qT = qT_pool.tile([128, NT], f16, tag="qT")
                if USE_FP8 and j == 0:
                    proj_qk_fused16(xt16, qT[:], kT_all[:, t0:t0 + NT])
                else:
                    proj_qk("q", j, xt16, xt8, qT[:])
                    proj_qk("k", j, xt16, xt8, kT_all[:, t0:t0 + NT])
                if USE_FP8 and j >= 1 and j + 1 < N_TT:
                    xt8_tiles[j + 1] = cast_x8(xt16_tiles[j + 1])
                if j + 2 < N_TT:
                    xt16_tiles[j + 2] = load_x16(j + 2)
                ex_all = ex_pool.tile([128, N_SC * NT], f16, tag="ex")
                n_sc = (j + 1) * N_TC
                if not last:
                    scores_exp(j, qT, ex_all, 0, n_sc)
                    v_proj_transpose(j, xt16)
                    # PV runs one tile behind: the in-order PE then fills
                    # this tile's exp-chain wait with the next tile's
                    # projections instead of stalling on PV's last-chunk
                    # dependency.
                    if prev is not None:
                        pv_out(*prev)
                    prev = (j, ex_all)
                else:
                    # last tile: no next-tile work exists to hide the PV
                    # tail, so interleave its PV chunks into the scores
                    # stream as their diagonal ex chunks become ready.
                    v_proj_transpose(j, xt16)
                    scores_exp(j, qT, ex_all, 0, j * N_TC + 2)
                    if prev is not None:
                        pv_out(*prev)
                    for tch in range(N_TC):
                        if j * N_TC + tch + 2 <= n_sc - 1:
                            scores_exp(j, qT, ex_all, j * N_TC + tch + 2,
                                       j * N_TC + tch + 3)
                        pv_chunk(j, ex_all, tch)
        else:
            # phase 1: all projections; phase 2: attention per t-tile
            for j in range(N_TT):
                t0 = j * NT
                xt16 = xt16_tiles.pop(j)
                xt8 = xt8_tiles.pop(j, None)
                proj_qk("q", j, xt16, xt8, qT_all[:, t0:t0 + NT])
                proj_qk("k", j, xt16, xt8, kT_all[:, t0:t0 + NT])
                v_proj_transpose(j, xt16)
                if USE_FP8 and j >= 1 and j + 1 < N_TT:
                    xt8_tiles[j + 1] = cast_x8(xt16_tiles[j + 1])
                if j + 2 < N_TT:
                    xt16_tiles[j + 2] = load_x16(j + 2)
            for j in range(N_TT):
                ex_all = ex_pool.tile([128, N_SC * NT], f16, tag="ex")
                scores_exp(j, qT_all[:, j * NT:(j + 1) * NT], ex_all,
                           0, N_SC)
                pv_out(j, ex_all)

    nc.compile()
    return nc


def _get(causal: bool):
    if causal not in _cache:
        _cache[causal] = _build(causal)
    return _cache[causal]


def _pack_x(xb, np_dtype):
    # [T, E] -> [128, (tile, chunk, NT)]: x[t, e] at
    # [e % 128, tile(t), chunk(e), t % NT]
    p = xb.reshape(-1, NT, N_EC, 128).transpose(3, 0, 2, 1)
    return np.ascontiguousarray(p.astype(np_dtype).reshape(128, -1))


def _pack_w(w, np_dtype):
    # [E, D] -> [128, (chunk, D)]: W[e, d] at [e % 128, chunk(e), d]
    p = w.reshape(N_EC, 128, D).transpose(1, 0, 2)
    return np.ascontiguousarray(p.astype(np_dtype).reshape(128, -1))


def _make_in_maps(x, Wq, bq, Wk, bk, Wv, bv):
    x = np.asarray(x, dtype=np.float32)
    Wq_s = np.asarray(Wq, np.float32) * WSCALE
    Wk_s = np.asarray(Wk, np.float32) * WSCALE
    shared = {
        "Wq16": _pack_w(Wq_s, np.float16),
        "Wk16": _pack_w(Wk_s, np.float16),
        "Wv16": _pack_w(np.asarray(Wv, np.float32), np.float16),
        "bq": np.ascontiguousarray(
            np.asarray(bq, np.float32).reshape(D, 1) * WSCALE),
        "bk": np.ascontiguousarray(
            np.asarray(bk, np.float32).reshape(D, 1) * WSCALE),
        "bv": np.ascontiguousarray(np.asarray(bv, np.float32).reshape(D, 1)),
    }
    f8 = None
    if USE_FP8:
        import ml_dtypes
        f8 = ml_dtypes.float8_e4m3
        shared["Wq8"] = _pack_w(Wq_s, f8)
        shared["Wk8"] = _pack_w(Wk_s, f8)
    in_maps = []
    for b in range(B):
        m = dict(shared)
        m["xP16"] = _pack_x(x[b], np.float16)
        if USE_FP8:
            m["xP8"] = _pack_x(x[b][NT:2 * NT], f8)
        in_maps.append(m)
    return in_maps


def kernel(x, Wq, bq, Wk, bk, Wv, bv, mask, **_ignored):
    from concourse.bass_utils import run_bass_kernel_spmd

    causal = bool(np.asarray(mask).item()) if mask is not None else False
    nc = _get(causal)
    in_maps = _make_in_maps(x, Wq, bq, Wk, bk, Wv, bv)
    res = run_bass_kernel_spmd(nc, in_maps, core_ids=list(range(B)))
    return np.stack([res.results[b]["out"] for b in range(B)],
                    axis=0).astype(np.float32)


# revision 19
# speedup vs baseline: 1.2968x; 1.2968x over previous
"""Causal single-head attention on 8 TRN2 NeuronCores.

Problem: x[B=8,T=2048,E=1024] fp32, per-head Q/K/V projections (D=128) +
causal softmax attention. Sharding: data-parallel over batch B — one batch
element per core; Q/K/V weights replicated.

Key structure (v3):
  - All inputs host-packed to the exact SBUF layout so every DMA is a
    contiguous [128, L] transfer: x as [128, (tile, chunk, 512)] in fp16
    and fp8, weights as [128, (chunk, 128)].
  - Q/K projections in fp8 e4m3 + DoubleRow (contraction 256/matmul) for
    t-tiles 1-3; t-tile 0 runs them in fp16 because its rows have few
    softmax terms, so q/k quantization noise doesn't average out there
    (it also produces k for s<512 in fp16, which every later tile reuses).
    V is always fp16: v errors hit the output directly.
    Wq/Wk/bq/bk pre-scaled by 32 on host (else ~50% of W mass is
    e4m3-subnormal); exp() scale absorbs 1/32^2.
  - Short PE warmup on the identity tile spans the initial DMA wait so
    HAM reaches 8/8 before the real matmuls.
  - Scores transposed S^T [s,128, t 512] = kT_slice @ qT, exp on ACT,
    diagonal masked by one 128x128 lower-triangle multiply; s-chunks
    above the diagonal skipped.
  - PV natural per 128-row t-chunk with ones-augmented v (denominator
    rides in psum column 128); PV runs one t-tile behind the scores so
    the exp chain never stalls the PE — except the last tile, whose PV
    chunks interleave into the scores stream to shorten the serial tail.
"""

import numpy as np

B, T, E, D = 8, 2048, 1024, 128
NT = 512                 # t-tile width (PSUM bank = 512 fp32)
N_TT = T // NT           # 4 t-tiles
N_TC = NT // 128         # 4 t-chunks per t-tile
N_EC = E // 128          # 8 e-chunks
N_SC = T // 128          # 16 s-chunks
VS = 132                 # v_aug free stride (129 used)
USE_FP8 = True
WSCALE = 32.0 if USE_FP8 else 1.0  # host pre-scale on Wq/Wk/bq/bk
SCALE = float(1.0 / (WSCALE * WSCALE * np.sqrt(D)))
N_WARM = 8

_cache: dict = {}


def _build(causal: bool):
    from contextlib import ExitStack
    import concourse.bass as bass
    import concourse.tile as tile
    from concourse import bacc, mybir
    from concourse.masks import make_identity

    f32 = mybir.dt.float32
    f16 = mybir.dt.float16
    f8 = mybir.dt.float8e4
    AF = mybir.ActivationFunctionType
    DR = mybir.MatmulPerfMode.DoubleRow

    nc = bacc.Bacc("TRN2", target_bir_lowering=False, debug=False,
                   num_devices=B)
    # packed inputs: [128, (tile, chunk, NT)] with x[t, e] at
    # [e % 128, tile(t), chunk(e), t % NT]
    xP16 = nc.dram_tensor("xP16", (128, N_TT * N_EC * NT), f16,
                          kind="ExternalInput").ap()
    Wq16 = nc.dram_tensor("Wq16", (128, N_EC * D), f16, kind="ExternalInput").ap()
    Wk16 = nc.dram_tensor("Wk16", (128, N_EC * D), f16, kind="ExternalInput").ap()
    Wv16 = nc.dram_tensor("Wv16", (128, N_EC * D), f16, kind="ExternalInput").ap()
    xP8 = Wq8 = Wk8 = None
    if USE_FP8:
        xP8 = nc.dram_tensor("xP8", (128, (N_TT - 1) * N_EC * NT), f8,
                             kind="ExternalInput").ap()
        Wq8 = nc.dram_tensor("Wq8", (128, N_EC * D), f8, kind="ExternalInput").ap()
        Wk8 = nc.dram_tensor("Wk8", (128, N_EC * D), f8, kind="ExternalInput").ap()
    bs = {p: nc.dram_tensor(f"b{p}", (D, 1), f32, kind="ExternalInput").ap()
          for p in "qkv"}
    out = nc.dram_tensor("out", (T, D), f16, kind="ExternalOutput").ap()

    with tile.TileContext(nc) as tc, ExitStack() as ctx:
        consts = ctx.enter_context(tc.tile_pool(name="consts", bufs=1))
        xt8_pool = ctx.enter_context(tc.tile_pool(name="xt8", bufs=3))
        xt16_pool = ctx.enter_context(tc.tile_pool(name="xt16", bufs=3))
        qT_pool = ctx.enter_context(tc.tile_pool(name="qT", bufs=2))
        vT_pool = ctx.enter_context(tc.tile_pool(name="vT", bufs=2))
        ex_pool = ctx.enter_context(tc.tile_pool(name="ex", bufs=3))
        small = ctx.enter_context(tc.tile_pool(name="small", bufs=8))
        outp = ctx.enter_context(tc.tile_pool(name="outp", bufs=8))
        ps_qkv = ctx.enter_context(tc.tile_pool(name="ps_qkv", bufs=2,
                                                space="PSUM"))
        ps_s = ctx.enter_context(tc.tile_pool(name="ps_s", bufs=3,
                                              space="PSUM"))
        ps_o = ctx.enter_context(tc.tile_pool(name="ps_o", bufs=2,
                                              space="PSUM"))
        ps_t = ctx.enter_context(tc.tile_pool(name="ps_t", bufs=1,
                                              space="PSUM"))

        # ---- DMAs: scalar ring carries weights, sync ring carries x ----
        # tile 0 computes q/k in fp16, so its critical path is Wq16+xt16_0.
        w16 = {p: consts.tile([128, N_EC, D], f16, tag=f"w16{p}",
                              name=f"w16{p}")
               for p in "qkv"}
        nc.scalar.dma_start(w16["q"][:].rearrange("p c d -> p (c d)"), Wq16[:])

        def load_x(pool, tag, dram, jj, dt, eng, pieces=1):
            # an x-tile as `pieces` separate tiles so dependencies unlock
            # per piece (tile deps are whole-tile, not subtile); each
            # piece is itself two chunk-range DMA transfers
            w = N_EC // pieces
            ph = []
            for h in range(pieces):
                xt = pool.tile([128, w, NT], dt, tag=f"{tag}{h}",
                               name=f"{tag}{h}")
                for g in range(2):
                    c0 = w * h + (w // 2) * g
                    t0 = (jj * N_EC + c0) * NT
                    eng.dma_start(
                        xt[:, (w // 2) * g:(w // 2) * (g + 1), :]
                        .rearrange("p c n -> p (c n)"),
                        dram[:, t0:t0 + (w // 2) * NT])
                ph.append((xt, w * h, w))
            return ph

        def xch(handle, c):
            # chunk accessor over a piece-list x handle
            for xt, c0, w in handle:
                if c0 <= c < c0 + w:
                    return xt[:, c - c0, :]
            raise KeyError(c)

        def xch2(handle, c):
            # two-chunk slice (DoubleRow) — never crosses a piece boundary
            for xt, c0, w in handle:
                if c0 <= 2 * c and 2 * c + 2 <= c0 + w:
                    return xt[:, 2 * c - c0:2 * c - c0 + 2, :]
            raise KeyError(c)

        def load_x16(jj):
            return load_x(xt16_pool, "xt16", xP16, jj, f16, nc.sync)

        def load_x8(jj):
            # fp8 x for t-tiles 1..3 (tile 0 is fp16-only); xP8 holds
            # tiles 1..3 back to back
            return load_x(xt8_pool, "xt8", xP8, jj - 1, f8, nc.sync)

        xt16_tiles = {0: load_x(xt16_pool, "xt16q", xP16, 0, f16, nc.sync,
                                pieces=4)}
        nc.scalar.dma_start(w16["k"][:].rearrange("p c d -> p (c d)"), Wk16[:])
        b_t = {}
        for p in "qkv":
            bt = consts.tile([128, 1], f32, tag=f"b{p}")
            nc.scalar.dma_start(bt[:], bs[p])
            b_t[p] = bt
        nc.scalar.dma_start(w16["v"][:].rearrange("p c d -> p (c d)"), Wv16[:])
        w8 = {}
        if USE_FP8:
            for p, W8 in (("q", Wq8), ("k", Wk8)):
                wt = consts.tile([128, N_EC, D], f8, tag=f"w8{p}")
                nc.scalar.dma_start(wt[:].rearrange("p c d -> p (c d)"), W8[:])
                w8[p] = wt
            xt8_tiles = {1: load_x8(1)} if N_TT > 1 else {}
        else:
            xt8_tiles = {}
        if N_TT > 1:
            xt16_tiles[1] = load_x16(1)

        ident_h = consts.tile([128, 128], f16, tag="ident_h")
        make_identity(nc, ident_h[:])

        # PE warmup while the first x tile lands: HAM needs ~3.4us of
        # sustained PE activity to lift the clock gate to 8/8.  Routing
        # every warmup matmul through the single ps_t buffer serializes
        # them on the WAW hazard (~0.4us apiece), stretching a handful of
        # matmuls across the whole DMA wait.
        warm_w = consts.tile([128, NT], f16, tag="warm_w")
        nc.vector.memset(warm_w[:], 0.0)
        for _ in range(N_WARM):
            pw = ps_s.tile([128, NT], f32, tag="ps_s")
            nc.tensor.matmul(pw[:], ident_h[:], warm_w[:],
                             start=True, stop=True)

        masks_h = None
        if causal:
            # single lower-triangular (keep t>=s) 128x128 block
            masks_h = consts.tile([128, 128], f16, tag="masks_h")
            nc.gpsimd.memset(masks_h[:], 1.0)
            nc.gpsimd.affine_select(
                out=masks_h[:], in_=masks_h[:],
                compare_op=mybir.AluOpType.is_ge,
                fill=0.0, base=0, channel_multiplier=-1,
                pattern=[[1, 128]])

        kT_all = consts.tile([128, T], f16, tag="kT_all")
        v_all = consts.tile([128, N_SC, VS], f16, tag="v_all")
        nc.vector.memset(v_all[:, :, 128:129], 1.0)  # the ones columns

        qT_all = None
        if not causal:
            # full attention needs every t-tile's q resident before phase 2
            qT_all = consts.tile([128, T], f16, tag="qT_all")

        def proj(w16_t, w8_t, xt16, xt8, dest, bias, fp8):
            ps = ps_qkv.tile([128, NT], f32, tag="ps_qkv")
            if fp8:
                for c in range(N_EC // 2):
                    nc.tensor.matmul(
                        ps[:], w8_t[:, 2 * c:2 * c + 2, :], xch2(xt8, c),
                        start=(c == 0), stop=(c == N_EC // 2 - 1),
                        perf_mode=DR)
            else:
                for c in range(N_EC):
                    nc.tensor.matmul(
                        ps[:], w16_t[:, c, :], xch(xt16, c),
                        start=(c == 0), stop=(c == N_EC - 1))
            nc.vector.tensor_scalar_add(dest, ps[:], bias[:])

        def proj_qk(p, j, xt16, xt8, dest):
            fp8 = USE_FP8 and j > 0
            proj(w16[p], w8.get(p), xt16, xt8, dest, b_t[p], fp8)

        def proj_qk_fused16(xt16, qdest, kdest):
            # tile 0 only: q and k accumulate interleaved into two psum
            # banks so each arriving x piece feeds 4 back-to-back matmuls
            # (piece arrival paces the PE here; sequential q-then-k would
            # outrun the DMA on q and then replay k from sbuf cold)
            psq = ps_qkv.tile([128, NT], f32, tag="ps_qkv")
            psk = ps_qkv.tile([128, NT], f32, tag="ps_qkv")
            for c in range(N_EC):
                xc = xch(xt16, c)
                nc.tensor.matmul(psq[:], w16["q"][:, c, :], xc,
                                 start=(c == 0), stop=(c == N_EC - 1),
                                 skip_group_check=True)
                nc.tensor.matmul(psk[:], w16["k"][:, c, :], xc,
                                 start=(c == 0), stop=(c == N_EC - 1),
                                 skip_group_check=True)
            nc.vector.tensor_scalar_add(qdest, psq[:], b_t["q"][:])
            nc.vector.tensor_scalar_add(kdest, psk[:], b_t["k"][:])

        def scores_exp(j, qT, ex_all, i0, i1):
            # Diagonal s-chunk m: columns t_local < 128*m are never read by
            # PV (those t-chunks exclude this s-chunk), so compute only
            # [128*m:NT] and mask just the 128-wide diagonal sub-block.
            for i in range(i0, i1):
                m = i - j * N_TC
                off = 128 * m if (causal and m > 0) else 0
                ps = ps_s.tile([128, NT], f32, tag="ps_s")
                nc.tensor.matmul(ps[:, off:NT],
                                 kT_all[:, i * 128:(i + 1) * 128],
                                 qT[:, off:NT], start=True, stop=True)
                ex = ex_all[:, i * NT + off:(i + 1) * NT]
                nc.scalar.activation(ex, ps[:, off:NT], AF.Exp, scale=SCALE)
                if causal and m >= 0:
                    nc.vector.tensor_mul(
                        ex_all[:, i * NT + off:i * NT + off + 128],
                        ex_all[:, i * NT + off:i * NT + off + 128],
                        masks_h[:])

        def v_proj_transpose(j, xt16):
            vT = vT_pool.tile([128, NT], f16, tag="vT")
            proj(w16["v"], None, xt16, None, vT[:], b_t["v"], False)
            for tch in range(N_TC):
                sc = j * N_TC + tch
                pt = ps_t.tile([128, 256], f16, tag="ps_t")
                nc.tensor.transpose(pt[:, 0:128],
                                    vT[:, tch * 128:(tch + 1) * 128],
                                    ident_h[:])
                nc.vector.tensor_copy(v_all[:, sc, 0:128], pt[:, 0:128])

        def pv_chunk(j, ex_all, tch):
            # PV natural for one t-chunk; denominator rides in column 128
            tc_glob = j * N_TC + tch
            n_i = tc_glob + 1 if causal else N_SC
            po = ps_o.tile([128, VS], f32, tag="ps_o")
            for i in range(n_i):
                nc.tensor.matmul(
                    po[:, 0:129],
                    ex_all[:, i * NT + tch * 128:i * NT + (tch + 1) * 128],
                    v_all[:, i, 0:129],
                    start=(i == 0), stop=(i == n_i - 1),
                    skip_group_check=True)
            rec = small.tile([128, 1], f32, tag="rec")
            nc.vector.reciprocal(rec[:], po[:, 128:129])
            ot = outp.tile([128, 128], f16, tag="ot")
            nc.vector.tensor_scalar_mul(ot[:], po[:, 0:128], rec[:])
            r0 = tc_glob * 128
            nc.sync.dma_start(out[r0:r0 + 128, :], ot[:])

        def pv_out(j, ex_all):
            for tch in range(N_TC):
                pv_chunk(j, ex_all, tch)

        if causal:
            prev = None
            for j in range(N_TT):
                t0 = j * NT
                last = j == N_TT - 1
                xt16 = xt16_tiles.pop(j)
                xt8 = xt8_tiles.pop(j, None)
                qT = qT_pool.tile([128, NT], f16, tag="qT")
                if USE_FP8 and j == 0:
                    proj_qk_fused16(xt16, qT[:], kT_all[:, t0:t0 + NT])
                else:
                    proj_qk("q", j, xt16, xt8, qT[:])
                    proj_qk("k", j, xt16, xt8, kT_all[:, t0:t0 + NT])
                if j + 2 < N_TT:
                    xt16_tiles[j + 2] = load_x16(j + 2)
                    if USE_FP8:
                        xt8_tiles[j + 2] = load_x8(j + 2)
                ex_all = ex_pool.tile([128, N_SC * NT], f16, tag="ex")
                n_sc = (j + 1) * N_TC
                if not last:
                    scores_exp(j, qT, ex_all, 0, n_sc)
                    v_proj_transpose(j, xt16)
                    # PV runs one tile behind: the in-order PE then fills
                    # this tile's exp-chain wait with the next tile's
                    # projections instead of stalling on PV's last-chunk
                    # dependency.
                    if prev is not None:
                        pv_out(*prev)
                    prev = (j, ex_all)
                else:
                    # last tile: no next-tile work exists to hide the PV
                    # tail, so interleave its PV chunks into the scores
                    # stream as their diagonal ex chunks become ready.
                    v_proj_transpose(j, xt16)
                    scores_exp(j, qT, ex_all, 0, j * N_TC + 2)
                    if prev is not None:
                        pv_out(*prev)
                    for tch in range(N_TC):
                        if j * N_TC + tch + 2 <= n_sc - 1:
                            scores_exp(j, qT, ex_all, j * N_TC + tch + 2,
                                       j * N_TC + tch + 3)
                        pv_chunk(j, ex_all, tch)
        else:
            # phase 1: all projections; phase 2: attention per t-tile
            for j in range(N_TT):
                t0 = j * NT
                xt16 = xt16_tiles.pop(j)
                xt8 = xt8_tiles.pop(j, None)
                proj_qk("q", j, xt16, xt8, qT_all[:, t0:t0 + NT])
                proj_qk("k", j, xt16, xt8, kT_all[:, t0:t0 + NT])
                v_proj_transpose(j, xt16)
                if j + 2 < N_TT:
                    xt16_tiles[j + 2] = load_x16(j + 2)
                    if USE_FP8:
                        xt8_tiles[j + 2] = load_x8(j + 2)
            for j in range(N_TT):
                ex_all = ex_pool.tile([128, N_SC * NT], f16, tag="ex")
                scores_exp(j, qT_all[:, j * NT:(j + 1) * NT], ex_all,
                           0, N_SC)
                pv_out(j, ex_all)

    nc.compile()
    return nc


def _get(causal: bool):
    if causal not in _cache:
        _cache[causal] = _build(causal)
    return _cache[causal]


def _pack_x(xb, np_dtype):
    # [T, E] -> [128, (tile, chunk, NT)]: x[t, e] at
    # [e % 128, tile(t), chunk(e), t % NT]
    p = xb.reshape(-1, NT, N_EC, 128).transpose(3, 0, 2, 1)
    return np.ascontiguousarray(p.astype(np_dtype).reshape(128, -1))


def _pack_w(w, np_dtype):
    # [E, D] -> [128, (chunk, D)]: W[e, d] at [e % 128, chunk(e), d]
    p = w.reshape(N_EC, 128, D).transpose(1, 0, 2)
    return np.ascontiguousarray(p.astype(np_dtype).reshape(128, -1))


def _make_in_maps(x, Wq, bq, Wk, bk, Wv, bv):
    x = np.asarray(x, dtype=np.float32)
    Wq_s = np.asarray(Wq, np.float32) * WSCALE
    Wk_s = np.asarray(Wk, np.float32) * WSCALE
    shared = {
        "Wq16": _pack_w(Wq_s, np.float16),
        "Wk16": _pack_w(Wk_s, np.float16),
        "Wv16": _pack_w(np.asarray(Wv, np.float32), np.float16),
        "bq": np.ascontiguousarray(
            np.asarray(bq, np.float32).reshape(D, 1) * WSCALE),
        "bk": np.ascontiguousarray(
            np.asarray(bk, np.float32).reshape(D, 1) * WSCALE),
        "bv": np.ascontiguousarray(np.asarray(bv, np.float32).reshape(D, 1)),
    }
    f8 = None
    if USE_FP8:
        import ml_dtypes
        f8 = ml_dtypes.float8_e4m3
        shared["Wq8"] = _pack_w(Wq_s, f8)
        shared["Wk8"] = _pack_w(Wk_s, f8)
    in_maps = []
    for b in range(B):
        m = dict(shared)
        m["xP16"] = _pack_x(x[b], np.float16)
        if USE_FP8:
            m["xP8"] = _pack_x(x[b][NT:], f8)
        in_maps.append(m)
    return in_maps


def kernel(x, Wq, bq, Wk, bk, Wv, bv, mask, **_ignored):
    from concourse.bass_utils import run_bass_kernel_spmd

    causal = bool(np.asarray(mask).item()) if mask is not None else False
    nc = _get(causal)
    in_maps = _make_in_maps(x, Wq, bq, Wk, bk, Wv, bv)
    res = run_bass_kernel_spmd(nc, in_maps, core_ids=list(range(B)))
    return np.stack([res.results[b]["out"] for b in range(B)],
                    axis=0).astype(np.float32)


# revision 20
# speedup vs baseline: 1.3857x; 1.0685x over previous
"""Causal single-head attention on 8 TRN2 NeuronCores.

Problem: x[B=8,T=2048,E=1024] fp32, per-head Q/K/V projections (D=128) +
causal softmax attention. Sharding: data-parallel over batch B — one batch
element per core; Q/K/V weights replicated.

Key structure (v3):
  - All inputs host-packed to the exact SBUF layout so every DMA is a
    contiguous [128, L] transfer: x as [128, (tile, chunk, 512)] in fp16
    and fp8, weights as [128, (chunk, 128)].
  - Q/K projections in fp8 e4m3 + DoubleRow (contraction 256/matmul) for
    t-tiles 1-3; t-tile 0 runs them in fp16 because its rows have few
    softmax terms, so q/k quantization noise doesn't average out there
    (it also produces k for s<512 in fp16, which every later tile reuses).
    V is always fp16: v errors hit the output directly.
    Wq/Wk/bq/bk pre-scaled by 32 on host (else ~50% of W mass is
    e4m3-subnormal); exp() scale absorbs 1/32^2.
  - Short PE warmup on the identity tile spans the initial DMA wait so
    HAM reaches 8/8 before the real matmuls.
  - Scores transposed S^T [s,128, t 512] = kT_slice @ qT, exp on ACT,
    diagonal masked by one 128x128 lower-triangle multiply; s-chunks
    above the diagonal skipped.
  - PV natural per 128-row t-chunk with ones-augmented v (denominator
    rides in psum column 128); PV runs one t-tile behind the scores so
    the exp chain never stalls the PE — except the last tile, whose PV
    chunks interleave into the scores stream to shorten the serial tail.
"""

import numpy as np

B, T, E, D = 8, 2048, 1024, 128
NT = 512                 # t-tile width (PSUM bank = 512 fp32)
N_TT = T // NT           # 4 t-tiles
N_TC = NT // 128         # 4 t-chunks per t-tile
N_EC = E // 128          # 8 e-chunks
N_SC = T // 128          # 16 s-chunks
VS = 132                 # v_aug free stride (129 used)
USE_FP8 = True
WSCALE = 32.0 if USE_FP8 else 1.0  # host pre-scale on Wq/Wk/bq/bk
SCALE = float(1.0 / (WSCALE * WSCALE * np.sqrt(D)))
N_WARM = 6

_cache: dict = {}


def _build(causal: bool):
    from contextlib import ExitStack
    import concourse.bass as bass
    import concourse.tile as tile
    from concourse import bacc, mybir
    from concourse.masks import make_identity

    f32 = mybir.dt.float32
    f16 = mybir.dt.float16
    f8 = mybir.dt.float8e4
    AF = mybir.ActivationFunctionType
    DR = mybir.MatmulPerfMode.DoubleRow

    nc = bacc.Bacc("TRN2", target_bir_lowering=False, debug=False,
                   num_devices=B)
    # packed inputs: [128, (tile, chunk, NT)] with x[t, e] at
    # [e % 128, tile(t), chunk(e), t % NT]
    xP16 = nc.dram_tensor("xP16", (128, N_TT * N_EC * NT), f16,
                          kind="ExternalInput").ap()
    Wq16 = nc.dram_tensor("Wq16", (128, N_EC * D), f16, kind="ExternalInput").ap()
    Wk16 = nc.dram_tensor("Wk16", (128, N_EC * D), f16, kind="ExternalInput").ap()
    Wv16 = nc.dram_tensor("Wv16", (128, N_EC * D), f16, kind="ExternalInput").ap()
    xP8 = Wq8 = Wk8 = None
    if USE_FP8:
        xP8 = nc.dram_tensor("xP8", (128, (N_TT - 1) * N_EC * NT), f8,
                             kind="ExternalInput").ap()
        Wq8 = nc.dram_tensor("Wq8", (128, N_EC * D), f8, kind="ExternalInput").ap()
        Wk8 = nc.dram_tensor("Wk8", (128, N_EC * D), f8, kind="ExternalInput").ap()
    bs = {p: nc.dram_tensor(f"b{p}", (D, 1), f32, kind="ExternalInput").ap()
          for p in "qkv"}
    out = nc.dram_tensor("out", (T, D), f16, kind="ExternalOutput").ap()

    with tile.TileContext(nc) as tc, ExitStack() as ctx:
        consts = ctx.enter_context(tc.tile_pool(name="consts", bufs=1))
        xt8_pool = ctx.enter_context(tc.tile_pool(name="xt8", bufs=3))
        xt16_pool = ctx.enter_context(tc.tile_pool(name="xt16", bufs=3))
        qT_pool = ctx.enter_context(tc.tile_pool(name="qT", bufs=2))
        vT_pool = ctx.enter_context(tc.tile_pool(name="vT", bufs=2))
        ex_pool = ctx.enter_context(tc.tile_pool(name="ex", bufs=3))
        small = ctx.enter_context(tc.tile_pool(name="small", bufs=8))
        outp = ctx.enter_context(tc.tile_pool(name="outp", bufs=8))
        ps_qkv = ctx.enter_context(tc.tile_pool(name="ps_qkv", bufs=2,
                                                space="PSUM"))
        ps_s = ctx.enter_context(tc.tile_pool(name="ps_s", bufs=3,
                                              space="PSUM"))
        ps_o = ctx.enter_context(tc.tile_pool(name="ps_o", bufs=2,
                                              space="PSUM"))
        ps_t = ctx.enter_context(tc.tile_pool(name="ps_t", bufs=1,
                                              space="PSUM"))

        # ---- DMAs: scalar ring carries weights, sync ring carries x ----
        # tile 0 computes q/k in fp16, so its critical path is Wq16+xt16_0.
        w16 = {p: consts.tile([128, N_EC, D], f16, tag=f"w16{p}",
                              name=f"w16{p}")
               for p in "qkv"}
        nc.scalar.dma_start(w16["q"][:].rearrange("p c d -> p (c d)"), Wq16[:])

        def load_x(pool, tag, dram, jj, dt, eng, pieces=1):
            # an x-tile as `pieces` separate tiles so dependencies unlock
            # per piece (tile deps are whole-tile, not subtile); each
            # piece is itself two chunk-range DMA transfers
            w = N_EC // pieces
            ph = []
            for h in range(pieces):
                xt = pool.tile([128, w, NT], dt, tag=f"{tag}{h}",
                               name=f"{tag}{h}")
                for g in range(2):
                    c0 = w * h + (w // 2) * g
                    t0 = (jj * N_EC + c0) * NT
                    eng.dma_start(
                        xt[:, (w // 2) * g:(w // 2) * (g + 1), :]
                        .rearrange("p c n -> p (c n)"),
                        dram[:, t0:t0 + (w // 2) * NT])
                ph.append((xt, w * h, w))
            return ph

        def xch(handle, c):
            # chunk accessor over a piece-list x handle
            for xt, c0, w in handle:
                if c0 <= c < c0 + w:
                    return xt[:, c - c0, :]
            raise KeyError(c)

        def xch2(handle, c):
            # two-chunk slice (DoubleRow) — never crosses a piece boundary
            for xt, c0, w in handle:
                if c0 <= 2 * c and 2 * c + 2 <= c0 + w:
                    return xt[:, 2 * c - c0:2 * c - c0 + 2, :]
            raise KeyError(c)

        def load_x16(jj):
            return load_x(xt16_pool, "xt16", xP16, jj, f16, nc.sync)

        def load_x8(jj):
            # fp8 x for t-tiles 1..3 (tile 0 is fp16-only); xP8 holds
            # tiles 1..3 back to back
            return load_x(xt8_pool, "xt8", xP8, jj - 1, f8, nc.sync)

        xt16_tiles = {0: load_x(xt16_pool, "xt16q", xP16, 0, f16, nc.sync,
                                pieces=4)}
        nc.scalar.dma_start(w16["k"][:].rearrange("p c d -> p (c d)"), Wk16[:])
        b_t = {}
        for p in "qkv":
            bt = consts.tile([128, 1], f32, tag=f"b{p}")
            nc.scalar.dma_start(bt[:], bs[p])
            b_t[p] = bt
        nc.scalar.dma_start(w16["v"][:].rearrange("p c d -> p (c d)"), Wv16[:])
        w8 = {}
        if USE_FP8:
            for p, W8 in (("q", Wq8), ("k", Wk8)):
                wt = consts.tile([128, N_EC, D], f8, tag=f"w8{p}")
                nc.scalar.dma_start(wt[:].rearrange("p c d -> p (c d)"), W8[:])
                w8[p] = wt
            xt8_tiles = {1: load_x8(1)} if N_TT > 1 else {}
        else:
            xt8_tiles = {}

        ident_h = consts.tile([128, 128], f16, tag="ident_h")
        make_identity(nc, ident_h[:])

        # PE warmup while the first x tile lands: HAM needs ~3.4us of
        # sustained PE activity to lift the clock gate to 8/8.  Routing
        # every warmup matmul through the single ps_t buffer serializes
        # them on the WAW hazard (~0.4us apiece), stretching a handful of
        # matmuls across the whole DMA wait.
        for _ in range(N_WARM):
            pw = ps_t.tile([128, 128], f32, tag="ps_t")
            nc.tensor.matmul(pw[:], ident_h[:], ident_h[:],
                             start=True, stop=True)

        masks_h = None
        if causal:
            # single lower-triangular (keep t>=s) 128x128 block
            masks_h = consts.tile([128, 128], f16, tag="masks_h")
            nc.gpsimd.memset(masks_h[:], 1.0)
            nc.gpsimd.affine_select(
                out=masks_h[:], in_=masks_h[:],
                compare_op=mybir.AluOpType.is_ge,
                fill=0.0, base=0, channel_multiplier=-1,
                pattern=[[1, 128]])

        kT_all = consts.tile([128, T], f16, tag="kT_all")
        v_all = consts.tile([128, N_SC, VS], f16, tag="v_all")
        nc.vector.memset(v_all[:, :, 128:129], 1.0)  # the ones columns

        qT_all = None
        if not causal:
            # full attention needs every t-tile's q resident before phase 2
            qT_all = consts.tile([128, T], f16, tag="qT_all")

        def proj(w16_t, w8_t, xt16, xt8, dest, bias, fp8):
            ps = ps_qkv.tile([128, NT], f32, tag="ps_qkv")
            if fp8:
                for c in range(N_EC // 2):
                    nc.tensor.matmul(
                        ps[:], w8_t[:, 2 * c:2 * c + 2, :], xch2(xt8, c),
                        start=(c == 0), stop=(c == N_EC // 2 - 1),
                        perf_mode=DR)
            else:
                for c in range(N_EC):
                    nc.tensor.matmul(
                        ps[:], w16_t[:, c, :], xch(xt16, c),
                        start=(c == 0), stop=(c == N_EC - 1))
            nc.vector.tensor_scalar_add(dest, ps[:], bias[:])

        def proj_qk(p, j, xt16, xt8, dest):
            fp8 = USE_FP8 and j > 0
            proj(w16[p], w8.get(p), xt16, xt8, dest, b_t[p], fp8)

        def proj_qk_fused16(xt16, qdest, kdest):
            # tile 0 only: q and k accumulate interleaved into two psum
            # banks so each arriving x piece feeds 4 back-to-back matmuls
            # (piece arrival paces the PE here; sequential q-then-k would
            # outrun the DMA on q and then replay k from sbuf cold)
            psq = ps_qkv.tile([128, NT], f32, tag="ps_qkv")
            psk = ps_qkv.tile([128, NT], f32, tag="ps_qkv")
            for c in range(N_EC):
                xc = xch(xt16, c)
                nc.tensor.matmul(psq[:], w16["q"][:, c, :], xc,
                                 start=(c == 0), stop=(c == N_EC - 1),
                                 skip_group_check=True)
                nc.tensor.matmul(psk[:], w16["k"][:, c, :], xc,
                                 start=(c == 0), stop=(c == N_EC - 1),
                                 skip_group_check=True)
            nc.vector.tensor_scalar_add(qdest, psq[:], b_t["q"][:])
            nc.vector.tensor_scalar_add(kdest, psk[:], b_t["k"][:])

        def scores_exp(j, qT, ex_all, i0, i1):
            # Diagonal s-chunk m: columns t_local < 128*m are never read by
            # PV (those t-chunks exclude this s-chunk), so compute only
            # [128*m:NT] and mask just the 128-wide diagonal sub-block.
            for i in range(i0, i1):
                m = i - j * N_TC
                off = 128 * m if (causal and m > 0) else 0
                ps = ps_s.tile([128, NT], f32, tag="ps_s")
                nc.tensor.matmul(ps[:, off:NT],
                                 kT_all[:, i * 128:(i + 1) * 128],
                                 qT[:, off:NT], start=True, stop=True)
                ex = ex_all[:, i * NT + off:(i + 1) * NT]
                nc.scalar.activation(ex, ps[:, off:NT], AF.Exp, scale=SCALE)
                if causal and m >= 0:
                    nc.vector.tensor_mul(
                        ex_all[:, i * NT + off:i * NT + off + 128],
                        ex_all[:, i * NT + off:i * NT + off + 128],
                        masks_h[:])

        def v_proj_transpose(j, xt16):
            vT = vT_pool.tile([128, NT], f16, tag="vT")
            proj(w16["v"], None, xt16, None, vT[:], b_t["v"], False)
            for tch in range(N_TC):
                sc = j * N_TC + tch
                pt = ps_t.tile([128, 256], f16, tag="ps_t")
                nc.tensor.transpose(pt[:, 0:128],
                                    vT[:, tch * 128:(tch + 1) * 128],
                                    ident_h[:])
                nc.vector.tensor_copy(v_all[:, sc, 0:128], pt[:, 0:128])

        def pv_chunk(j, ex_all, tch):
            # PV natural for one t-chunk; denominator rides in column 128
            tc_glob = j * N_TC + tch
            n_i = tc_glob + 1 if causal else N_SC
            po = ps_o.tile([128, VS], f32, tag="ps_o")
            for i in range(n_i):
                nc.tensor.matmul(
                    po[:, 0:129],
                    ex_all[:, i * NT + tch * 128:i * NT + (tch + 1) * 128],
                    v_all[:, i, 0:129],
                    start=(i == 0), stop=(i == n_i - 1),
                    skip_group_check=True)
            rec = small.tile([128, 1], f32, tag="rec")
            nc.vector.reciprocal(rec[:], po[:, 128:129])
            ot = outp.tile([128, 128], f16, tag="ot")
            nc.vector.tensor_scalar_mul(ot[:], po[:, 0:128], rec[:])
            r0 = tc_glob * 128
            nc.sync.dma_start(out[r0:r0 + 128, :], ot[:])

        def pv_out(j, ex_all):
            for tch in range(N_TC):
                pv_chunk(j, ex_all, tch)

        if causal:
            prev = None
            for j in range(N_TT):
                t0 = j * NT
                last = j == N_TT - 1
                xt16 = xt16_tiles.pop(j)
                xt8 = xt8_tiles.pop(j, None)
                qT = qT_pool.tile([128, NT], f16, tag="qT")
                if USE_FP8 and j == 0:
                    proj_qk_fused16(xt16, qT[:], kT_all[:, t0:t0 + NT])
                else:
                    proj_qk("q", j, xt16, xt8, qT[:])
                    proj_qk("k", j, xt16, xt8, kT_all[:, t0:t0 + NT])
                if j + 1 < N_TT:
                    xt16_tiles[j + 1] = load_x16(j + 1)
                    if USE_FP8 and j + 2 < N_TT:
                        xt8_tiles[j + 2] = load_x8(j + 2)
                ex_all = ex_pool.tile([128, N_SC * NT], f16, tag="ex")
                n_sc = (j + 1) * N_TC
                if not last:
                    scores_exp(j, qT, ex_all, 0, n_sc)
                    v_proj_transpose(j, xt16)
                    # PV runs one tile behind: the in-order PE then fills
                    # this tile's exp-chain wait with the next tile's
                    # projections instead of stalling on PV's last-chunk
                    # dependency.
                    if prev is not None:
                        pv_out(*prev)
                    prev = (j, ex_all)
                else:
                    # last tile: no next-tile work exists to hide the PV
                    # tail, so interleave its PV chunks into the scores
                    # stream as their diagonal ex chunks become ready.
                    v_proj_transpose(j, xt16)
                    scores_exp(j, qT, ex_all, 0, j * N_TC + 2)
                    if prev is not None:
                        pv_out(*prev)
                    for tch in range(N_TC):
                        if j * N_TC + tch + 2 <= n_sc - 1:
                            scores_exp(j, qT, ex_all, j * N_TC + tch + 2,
                                       j * N_TC + tch + 3)
                        pv_chunk(j, ex_all, tch)
        else:
            # phase 1: all projections; phase 2: attention per t-tile
            for j in range(N_TT):
                t0 = j * NT
                xt16 = xt16_tiles.pop(j)
                xt8 = xt8_tiles.pop(j, None)
                proj_qk("q", j, xt16, xt8, qT_all[:, t0:t0 + NT])
                proj_qk("k", j, xt16, xt8, kT_all[:, t0:t0 + NT])
                v_proj_transpose(j, xt16)
                if j + 1 < N_TT:
                    xt16_tiles[j + 1] = load_x16(j + 1)
                    if USE_FP8 and j + 2 < N_TT:
                        xt8_tiles[j + 2] = load_x8(j + 2)
            for j in range(N_TT):
                ex_all = ex_pool.tile([128, N_SC * NT], f16, tag="ex")
                scores_exp(j, qT_all[:, j * NT:(j + 1) * NT], ex_all,
                           0, N_SC)
                pv_out(j, ex_all)

    nc.compile()
    return nc


def _get(causal: bool):
    if causal not in _cache:
        _cache[causal] = _build(causal)
    return _cache[causal]


def _pack_x(xb, np_dtype):
    # [T, E] -> [128, (tile, chunk, NT)]: x[t, e] at
    # [e % 128, tile(t), chunk(e), t % NT]
    p = xb.reshape(-1, NT, N_EC, 128).transpose(3, 0, 2, 1)
    return np.ascontiguousarray(p.astype(np_dtype).reshape(128, -1))


def _pack_w(w, np_dtype):
    # [E, D] -> [128, (chunk, D)]: W[e, d] at [e % 128, chunk(e), d]
    p = w.reshape(N_EC, 128, D).transpose(1, 0, 2)
    return np.ascontiguousarray(p.astype(np_dtype).reshape(128, -1))


def _make_in_maps(x, Wq, bq, Wk, bk, Wv, bv):
    x = np.asarray(x, dtype=np.float32)
    Wq_s = np.asarray(Wq, np.float32) * WSCALE
    Wk_s = np.asarray(Wk, np.float32) * WSCALE
    shared = {
        "Wq16": _pack_w(Wq_s, np.float16),
        "Wk16": _pack_w(Wk_s, np.float16),
        "Wv16": _pack_w(np.asarray(Wv, np.float32), np.float16),
        "bq": np.ascontiguousarray(
            np.asarray(bq, np.float32).reshape(D, 1) * WSCALE),
        "bk": np.ascontiguousarray(
            np.asarray(bk, np.float32).reshape(D, 1) * WSCALE),
        "bv": np.ascontiguousarray(np.asarray(bv, np.float32).reshape(D, 1)),
    }
    f8 = None
    if USE_FP8:
        import ml_dtypes
        f8 = ml_dtypes.float8_e4m3
        shared["Wq8"] = _pack_w(Wq_s, f8)
        shared["Wk8"] = _pack_w(Wk_s, f8)
    in_maps = []
    for b in range(B):
        m = dict(shared)
        m["xP16"] = _pack_x(x[b], np.float16)
        if USE_FP8:
            m["xP8"] = _pack_x(x[b][NT:], f8)
        in_maps.append(m)
    return in_maps


def kernel(x, Wq, bq, Wk, bk, Wv, bv, mask, **_ignored):
    from concourse.bass_utils import run_bass_kernel_spmd

    causal = bool(np.asarray(mask).item()) if mask is not None else False
    nc = _get(causal)
    in_maps = _make_in_maps(x, Wq, bq, Wk, bk, Wv, bv)
    res = run_bass_kernel_spmd(nc, in_maps, core_ids=list(range(B)))
    return np.stack([res.results[b]["out"] for b in range(B)],
                    axis=0).astype(np.float32)


# revision 21
# speedup vs baseline: 1.4054x; 1.0142x over previous
"""Causal single-head attention on 8 TRN2 NeuronCores.

Problem: x[B=8,T=2048,E=1024] fp32, per-head Q/K/V projections (D=128) +
causal softmax attention. Sharding: data-parallel over batch B — one batch
element per core; Q/K/V weights replicated.

Key structure (measured ~54.5us vs 62.0us baseline; fixed framework
overhead alone is ~14us, so the compute window is ~40.5us with the PE
>90% occupied):
  - All inputs host-packed to the exact SBUF layout so every DMA is a
    contiguous [128, L] transfer: x as [128, (tile, chunk, 512)] in fp16
    and fp8, weights as [128, (chunk, 128)].  Tile-0's fp16 x arrives as
    four separate quarter tiles so projection matmuls start as soon as
    the first 256KB lands (tile deps are whole-tile, not subtile).
  - Q/K projections in fp8 e4m3 + DoubleRow (contraction 256/matmul,
    half the matmuls of fp16) for t-tiles 1-3; t-tile 0 runs them in
    fp16 because its rows have few softmax terms, so q/k quantization
    noise doesn't average out there (it also produces k for s<512 in
    fp16, which every later tile reuses); tile-0's q and k accumulations
    are interleaved into two psum banks so each arriving x quarter feeds
    four back-to-back matmuls.  V is always fp16: v errors hit the
    output directly (fp8 v measurably breaks the 2e-2 gate).
    Wq/Wk/bq/bk pre-scaled by 32 on host (else ~50% of W mass is
    e4m3-subnormal); exp() scale absorbs 1/32^2.
  - Short serialized PE warmup on the identity tile spans the initial
    DMA wait so the HAM clock gate lifts toward 8/8 before real matmuls.
  - Scores transposed S^T [s 128, t 512] = kT_slice @ qT, exp on ACT,
    diagonal masked by one 128x128 lower-triangle multiply; s-chunks
    above the diagonal skipped, diagonal chunks column-clipped.
  - PV natural per 128-row t-chunk with ones-augmented v (denominator
    rides in psum column 128); PV runs one t-tile behind the scores so
    the exp chain never stalls the PE — except the last tile, whose PV
    chunks interleave into the scores stream (scores one chunk ahead)
    to shorten the serial tail.  Output leaves as fp16 (cast back to
    f32 on host) to halve the output stream.
"""

import numpy as np

B, T, E, D = 8, 2048, 1024, 128
NT = 512                 # t-tile width (PSUM bank = 512 fp32)
N_TT = T // NT           # 4 t-tiles
N_TC = NT // 128         # 4 t-chunks per t-tile
N_EC = E // 128          # 8 e-chunks
N_SC = T // 128          # 16 s-chunks
VS = 132                 # v_aug free stride (129 used)
USE_FP8 = True
WSCALE = 32.0 if USE_FP8 else 1.0  # host pre-scale on Wq/Wk/bq/bk
SCALE = float(1.0 / (WSCALE * WSCALE * np.sqrt(D)))
N_WARM = 6

_cache: dict = {}


def _build(causal: bool):
    from contextlib import ExitStack
    import concourse.bass as bass
    import concourse.tile as tile
    from concourse import bacc, mybir
    from concourse.masks import make_identity

    f32 = mybir.dt.float32
    f16 = mybir.dt.float16
    f8 = mybir.dt.float8e4
    AF = mybir.ActivationFunctionType
    DR = mybir.MatmulPerfMode.DoubleRow

    nc = bacc.Bacc("TRN2", target_bir_lowering=False, debug=False,
                   num_devices=B)
    # packed inputs: [128, (tile, chunk, NT)] with x[t, e] at
    # [e % 128, tile(t), chunk(e), t % NT]
    xP16 = nc.dram_tensor("xP16", (128, N_TT * N_EC * NT), f16,
                          kind="ExternalInput").ap()
    Wq16 = nc.dram_tensor("Wq16", (128, N_EC * D), f16, kind="ExternalInput").ap()
    Wk16 = nc.dram_tensor("Wk16", (128, N_EC * D), f16, kind="ExternalInput").ap()
    Wv16 = nc.dram_tensor("Wv16", (128, N_EC * D), f16, kind="ExternalInput").ap()
    xP8 = Wq8 = Wk8 = None
    if USE_FP8:
        xP8 = nc.dram_tensor("xP8", (128, (N_TT - 1) * N_EC * NT), f8,
                             kind="ExternalInput").ap()
        Wq8 = nc.dram_tensor("Wq8", (128, N_EC * D), f8, kind="ExternalInput").ap()
        Wk8 = nc.dram_tensor("Wk8", (128, N_EC * D), f8, kind="ExternalInput").ap()
    bs = {p: nc.dram_tensor(f"b{p}", (D, 1), f32, kind="ExternalInput").ap()
          for p in "qkv"}
    out = nc.dram_tensor("out", (T, D), f16, kind="ExternalOutput").ap()

    with tile.TileContext(nc) as tc, ExitStack() as ctx:
        consts = ctx.enter_context(tc.tile_pool(name="consts", bufs=1))
        xt8_pool = ctx.enter_context(tc.tile_pool(name="xt8", bufs=3))
        xt16_pool = ctx.enter_context(tc.tile_pool(name="xt16", bufs=3))
        qT_pool = ctx.enter_context(tc.tile_pool(name="qT", bufs=2))
        vT_pool = ctx.enter_context(tc.tile_pool(name="vT", bufs=2))
        ex_pool = ctx.enter_context(tc.tile_pool(name="ex", bufs=3))
        small = ctx.enter_context(tc.tile_pool(name="small", bufs=8))
        outp = ctx.enter_context(tc.tile_pool(name="outp", bufs=8))
        ps_qkv = ctx.enter_context(tc.tile_pool(name="ps_qkv", bufs=2,
                                                space="PSUM"))
        ps_s = ctx.enter_context(tc.tile_pool(name="ps_s", bufs=3,
                                              space="PSUM"))
        ps_o = ctx.enter_context(tc.tile_pool(name="ps_o", bufs=2,
                                              space="PSUM"))
        ps_t = ctx.enter_context(tc.tile_pool(name="ps_t", bufs=1,
                                              space="PSUM"))

        # ---- DMAs: scalar ring carries weights, sync ring carries x ----
        # tile 0 computes q/k in fp16, so its critical path is Wq16+xt16_0.
        w16 = {p: consts.tile([128, N_EC, D], f16, tag=f"w16{p}",
                              name=f"w16{p}")
               for p in "qkv"}
        nc.scalar.dma_start(w16["q"][:].rearrange("p c d -> p (c d)"), Wq16[:])

        def load_x(pool, tag, dram, jj, dt, eng, pieces=1):
            # an x-tile as `pieces` separate tiles so dependencies unlock
            # per piece (tile deps are whole-tile, not subtile); each
            # piece is itself two chunk-range DMA transfers
            w = N_EC // pieces
            ph = []
            for h in range(pieces):
                xt = pool.tile([128, w, NT], dt, tag=f"{tag}{h}",
                               name=f"{tag}{h}")
                for g in range(2):
                    c0 = w * h + (w // 2) * g
                    t0 = (jj * N_EC + c0) * NT
                    eng.dma_start(
                        xt[:, (w // 2) * g:(w // 2) * (g + 1), :]
                        .rearrange("p c n -> p (c n)"),
                        dram[:, t0:t0 + (w // 2) * NT])
                ph.append((xt, w * h, w))
            return ph

        def xch(handle, c):
            # chunk accessor over a piece-list x handle
            for xt, c0, w in handle:
                if c0 <= c < c0 + w:
                    return xt[:, c - c0, :]
            raise KeyError(c)

        def xch2(handle, c):
            # two-chunk slice (DoubleRow) — never crosses a piece boundary
            for xt, c0, w in handle:
                if c0 <= 2 * c and 2 * c + 2 <= c0 + w:
                    return xt[:, 2 * c - c0:2 * c - c0 + 2, :]
            raise KeyError(c)

        def load_x16(jj):
            return load_x(xt16_pool, "xt16", xP16, jj, f16, nc.sync)

        def load_x8(jj):
            # fp8 x for t-tiles 1..3 (tile 0 is fp16-only); xP8 holds
            # tiles 1..3 back to back
            return load_x(xt8_pool, "xt8", xP8, jj - 1, f8, nc.sync)

        xt16_tiles = {0: load_x(xt16_pool, "xt16q", xP16, 0, f16, nc.sync,
                                pieces=4)}
        nc.scalar.dma_start(w16["k"][:].rearrange("p c d -> p (c d)"), Wk16[:])
        b_t = {}
        for p in "qkv":
            bt = consts.tile([128, 1], f32, tag=f"b{p}")
            nc.scalar.dma_start(bt[:], bs[p])
            b_t[p] = bt
        nc.scalar.dma_start(w16["v"][:].rearrange("p c d -> p (c d)"), Wv16[:])
        w8 = {}
        if USE_FP8:
            for p, W8 in (("q", Wq8), ("k", Wk8)):
                wt = consts.tile([128, N_EC, D], f8, tag=f"w8{p}")
                nc.scalar.dma_start(wt[:].rearrange("p c d -> p (c d)"), W8[:])
                w8[p] = wt
            xt8_tiles = {1: load_x8(1)} if N_TT > 1 else {}
        else:
            xt8_tiles = {}

        ident_h = consts.tile([128, 128], f16, tag="ident_h")
        make_identity(nc, ident_h[:])

        # PE warmup while the first x tile lands: HAM needs ~3.4us of
        # sustained PE activity to lift the clock gate to 8/8.  Routing
        # every warmup matmul through the single ps_t buffer serializes
        # them on the WAW hazard (~0.4us apiece), stretching a handful of
        # matmuls across the whole DMA wait.
        for _ in range(N_WARM):
            pw = ps_t.tile([128, 128], f32, tag="ps_t")
            nc.tensor.matmul(pw[:], ident_h[:], ident_h[:],
                             start=True, stop=True)

        masks_h = None
        if causal:
            # single lower-triangular (keep t>=s) 128x128 block
            masks_h = consts.tile([128, 128], f16, tag="masks_h")
            nc.gpsimd.memset(masks_h[:], 1.0)
            nc.gpsimd.affine_select(
                out=masks_h[:], in_=masks_h[:],
                compare_op=mybir.AluOpType.is_ge,
                fill=0.0, base=0, channel_multiplier=-1,
                pattern=[[1, 128]])

        kT_all = consts.tile([128, T], f16, tag="kT_all")
        v_all = consts.tile([128, N_SC, VS], f16, tag="v_all")
        nc.vector.memset(v_all[:, :, 128:129], 1.0)  # the ones columns

        qT_all = None
        if not causal:
            # full attention needs every t-tile's q resident before phase 2
            qT_all = consts.tile([128, T], f16, tag="qT_all")

        def proj(w16_t, w8_t, xt16, xt8, dest, bias, fp8):
            ps = ps_qkv.tile([128, NT], f32, tag="ps_qkv")
            if fp8:
                for c in range(N_EC // 2):
                    nc.tensor.matmul(
                        ps[:], w8_t[:, 2 * c:2 * c + 2, :], xch2(xt8, c),
                        start=(c == 0), stop=(c == N_EC // 2 - 1),
                        perf_mode=DR)
            else:
                for c in range(N_EC):
                    nc.tensor.matmul(
                        ps[:], w16_t[:, c, :], xch(xt16, c),
                        start=(c == 0), stop=(c == N_EC - 1))
            nc.vector.tensor_scalar_add(dest, ps[:], bias[:])

        def proj_qk(p, j, xt16, xt8, dest):
            fp8 = USE_FP8 and j > 0
            proj(w16[p], w8.get(p), xt16, xt8, dest, b_t[p], fp8)

        def proj_qk_fused16(xt16, qdest, kdest):
            # tile 0 only: q and k accumulate interleaved into two psum
            # banks so each arriving x piece feeds 4 back-to-back matmuls
            # (piece arrival paces the PE here; sequential q-then-k would
            # outrun the DMA on q and then replay k from sbuf cold)
            psq = ps_qkv.tile([128, NT], f32, tag="ps_qkv")
            psk = ps_qkv.tile([128, NT], f32, tag="ps_qkv")
            for c in range(N_EC):
                xc = xch(xt16, c)
                nc.tensor.matmul(psq[:], w16["q"][:, c, :], xc,
                                 start=(c == 0), stop=(c == N_EC - 1),
                                 skip_group_check=True)
                nc.tensor.matmul(psk[:], w16["k"][:, c, :], xc,
                                 start=(c == 0), stop=(c == N_EC - 1),
                                 skip_group_check=True)
            nc.vector.tensor_scalar_add(qdest, psq[:], b_t["q"][:])
            nc.vector.tensor_scalar_add(kdest, psk[:], b_t["k"][:])

        def scores_exp(j, qT, ex_all, i0, i1):
            # Diagonal s-chunk m: columns t_local < 128*m are never read by
            # PV (those t-chunks exclude this s-chunk), so compute only
            # [128*m:NT] and mask just the 128-wide diagonal sub-block.
            for i in range(i0, i1):
                m = i - j * N_TC
                off = 128 * m if (causal and m > 0) else 0
                ps = ps_s.tile([128, NT], f32, tag="ps_s")
                nc.tensor.matmul(ps[:, off:NT],
                                 kT_all[:, i * 128:(i + 1) * 128],
                                 qT[:, off:NT], start=True, stop=True)
                ex = ex_all[:, i * NT + off:(i + 1) * NT]
                nc.scalar.activation(ex, ps[:, off:NT], AF.Exp, scale=SCALE)
                if causal and m >= 0:
                    nc.vector.tensor_mul(
                        ex_all[:, i * NT + off:i * NT + off + 128],
                        ex_all[:, i * NT + off:i * NT + off + 128],
                        masks_h[:])

        def v_proj_transpose(j, xt16):
            vT = vT_pool.tile([128, NT], f16, tag="vT")
            proj(w16["v"], None, xt16, None, vT[:], b_t["v"], False)
            for tch in range(N_TC):
                sc = j * N_TC + tch
                pt = ps_t.tile([128, 256], f16, tag="ps_t")
                nc.tensor.transpose(pt[:, 0:128],
                                    vT[:, tch * 128:(tch + 1) * 128],
                                    ident_h[:])
                nc.vector.tensor_copy(v_all[:, sc, 0:128], pt[:, 0:128])

        def pv_chunk(j, ex_all, tch):
            # PV natural for one t-chunk; denominator rides in column 128
            tc_glob = j * N_TC + tch
            n_i = tc_glob + 1 if causal else N_SC
            po = ps_o.tile([128, VS], f32, tag="ps_o")
            for i in range(n_i):
                nc.tensor.matmul(
                    po[:, 0:129],
                    ex_all[:, i * NT + tch * 128:i * NT + (tch + 1) * 128],
                    v_all[:, i, 0:129],
                    start=(i == 0), stop=(i == n_i - 1),
                    skip_group_check=True)
            rec = small.tile([128, 1], f32, tag="rec")
            nc.vector.reciprocal(rec[:], po[:, 128:129])
            ot = outp.tile([128, 128], f16, tag="ot")
            nc.vector.tensor_scalar_mul(ot[:], po[:, 0:128], rec[:])
            r0 = tc_glob * 128
            nc.sync.dma_start(out[r0:r0 + 128, :], ot[:])

        def pv_out(j, ex_all):
            for tch in range(N_TC):
                pv_chunk(j, ex_all, tch)

        if causal:
            prev = None
            for j in range(N_TT):
                t0 = j * NT
                last = j == N_TT - 1
                xt16 = xt16_tiles.pop(j)
                xt8 = xt8_tiles.pop(j, None)
                qT = qT_pool.tile([128, NT], f16, tag="qT")
                if USE_FP8 and j == 0:
                    proj_qk_fused16(xt16, qT[:], kT_all[:, t0:t0 + NT])
                else:
                    proj_qk("q", j, xt16, xt8, qT[:])
                    proj_qk("k", j, xt16, xt8, kT_all[:, t0:t0 + NT])
                if j + 1 < N_TT:
                    xt16_tiles[j + 1] = load_x16(j + 1)
                    if USE_FP8 and j + 2 < N_TT:
                        xt8_tiles[j + 2] = load_x8(j + 2)
                ex_all = ex_pool.tile([128, N_SC * NT], f16, tag="ex")
                n_sc = (j + 1) * N_TC
                if not last:
                    scores_exp(j, qT, ex_all, 0, n_sc)
                    v_proj_transpose(j, xt16)
                    # PV runs one tile behind: the in-order PE then fills
                    # this tile's exp-chain wait with the next tile's
                    # projections instead of stalling on PV's last-chunk
                    # dependency.
                    if prev is not None:
                        pv_out(*prev)
                    prev = (j, ex_all)
                else:
                    # last tile: no next-tile work exists to hide the PV
                    # tail, so interleave its PV chunks into the scores
                    # stream as their diagonal ex chunks become ready.
                    v_proj_transpose(j, xt16)
                    scores_exp(j, qT, ex_all, 0, j * N_TC + 2)
                    if prev is not None:
                        pv_out(*prev)
                    for tch in range(N_TC):
                        if j * N_TC + tch + 2 <= n_sc - 1:
                            scores_exp(j, qT, ex_all, j * N_TC + tch + 2,
                                       j * N_TC + tch + 3)
                        pv_chunk(j, ex_all, tch)
        else:
            # phase 1: all projections; phase 2: attention per t-tile
            for j in range(N_TT):
                t0 = j * NT
                xt16 = xt16_tiles.pop(j)
                xt8 = xt8_tiles.pop(j, None)
                proj_qk("q", j, xt16, xt8, qT_all[:, t0:t0 + NT])
                proj_qk("k", j, xt16, xt8, kT_all[:, t0:t0 + NT])
                v_proj_transpose(j, xt16)
                if j + 1 < N_TT:
                    xt16_tiles[j + 1] = load_x16(j + 1)
                    if USE_FP8 and j + 2 < N_TT:
                        xt8_tiles[j + 2] = load_x8(j + 2)
            for j in range(N_TT):
                ex_all = ex_pool.tile([128, N_SC * NT], f16, tag="ex")
                scores_exp(j, qT_all[:, j * NT:(j + 1) * NT], ex_all,
                           0, N_SC)
                pv_out(j, ex_all)

    nc.compile()
    return nc


def _get(causal: bool):
    if causal not in _cache:
        _cache[causal] = _build(causal)
    return _cache[causal]


def _pack_x(xb, np_dtype):
    # [T, E] -> [128, (tile, chunk, NT)]: x[t, e] at
    # [e % 128, tile(t), chunk(e), t % NT]
    p = xb.reshape(-1, NT, N_EC, 128).transpose(3, 0, 2, 1)
    return np.ascontiguousarray(p.astype(np_dtype).reshape(128, -1))


def _pack_w(w, np_dtype):
    # [E, D] -> [128, (chunk, D)]: W[e, d] at [e % 128, chunk(e), d]
    p = w.reshape(N_EC, 128, D).transpose(1, 0, 2)
    return np.ascontiguousarray(p.astype(np_dtype).reshape(128, -1))


def _make_in_maps(x, Wq, bq, Wk, bk, Wv, bv):
    x = np.asarray(x, dtype=np.float32)
    Wq_s = np.asarray(Wq, np.float32) * WSCALE
    Wk_s = np.asarray(Wk, np.float32) * WSCALE
    shared = {
        "Wq16": _pack_w(Wq_s, np.float16),
        "Wk16": _pack_w(Wk_s, np.float16),
        "Wv16": _pack_w(np.asarray(Wv, np.float32), np.float16),
        "bq": np.ascontiguousarray(
            np.asarray(bq, np.float32).reshape(D, 1) * WSCALE),
        "bk": np.ascontiguousarray(
            np.asarray(bk, np.float32).reshape(D, 1) * WSCALE),
        "bv": np.ascontiguousarray(np.asarray(bv, np.float32).reshape(D, 1)),
    }
    f8 = None
    if USE_FP8:
        import ml_dtypes
        f8 = ml_dtypes.float8_e4m3
        shared["Wq8"] = _pack_w(Wq_s, f8)
        shared["Wk8"] = _pack_w(Wk_s, f8)
    in_maps = []
    for b in range(B):
        m = dict(shared)
        m["xP16"] = _pack_x(x[b], np.float16)
        if USE_FP8:
            m["xP8"] = _pack_x(x[b][NT:], f8)
        in_maps.append(m)
    return in_maps


def kernel(x, Wq, bq, Wk, bk, Wv, bv, mask, **_ignored):
    from concourse.bass_utils import run_bass_kernel_spmd

    causal = bool(np.asarray(mask).item()) if mask is not None else False
    nc = _get(causal)
    in_maps = _make_in_maps(x, Wq, bq, Wk, bk, Wv, bv)
    res = run_bass_kernel_spmd(nc, in_maps, core_ids=list(range(B)))
    return np.stack([res.results[b]["out"] for b in range(B)],
                    axis=0).astype(np.float32)


# revision 22
# speedup vs baseline: 1.4105x; 1.0036x over previous
"""Causal single-head attention on 8 TRN2 NeuronCores.

Problem: x[B=8,T=2048,E=1024] fp32, per-head Q/K/V projections (D=128) +
causal softmax attention. Sharding: data-parallel over batch B — one batch
element per core; Q/K/V weights replicated.

Key structure (measured ~54.5us vs 62.0us baseline; fixed framework
overhead alone is ~14us, so the compute window is ~40.5us with the PE
>90% occupied):
  - All inputs host-packed to the exact SBUF layout so every DMA is a
    contiguous [128, L] transfer: x as [128, (tile, chunk, 512)] in fp16
    and fp8, weights as [128, (chunk, 128)].  Tile-0's fp16 x arrives as
    four separate quarter tiles so projection matmuls start as soon as
    the first 256KB lands (tile deps are whole-tile, not subtile).
  - Q/K projections in fp8 e4m3 + DoubleRow (contraction 256/matmul,
    half the matmuls of fp16) for t-tiles 1-3; t-tile 0 runs them in
    fp16 because its rows have few softmax terms, so q/k quantization
    noise doesn't average out there (it also produces k for s<512 in
    fp16, which every later tile reuses); tile-0's q and k accumulations
    are interleaved into two psum banks so each arriving x quarter feeds
    four back-to-back matmuls.  V is always fp16: v errors hit the
    output directly (fp8 v measurably breaks the 2e-2 gate).
    Wq/Wk/bq/bk pre-scaled by 32 on host (else ~50% of W mass is
    e4m3-subnormal); exp() scale absorbs 1/32^2.
  - Short serialized PE warmup on the identity tile spans the initial
    DMA wait so the HAM clock gate lifts toward 8/8 before real matmuls.
  - Scores transposed S^T [s 128, t 512] = kT_slice @ qT, exp on ACT,
    diagonal masked by one 128x128 lower-triangle multiply; s-chunks
    above the diagonal skipped, diagonal chunks column-clipped.
  - PV natural per 128-row t-chunk with ones-augmented v (denominator
    rides in psum column 128); PV runs one t-tile behind the scores so
    the exp chain never stalls the PE — except the last tile, whose PV
    chunks interleave into the scores stream (scores one chunk ahead)
    to shorten the serial tail.  Output leaves as fp16 (cast back to
    f32 on host) to halve the output stream.
"""

import numpy as np

B, T, E, D = 8, 2048, 1024, 128
NT = 512                 # t-tile width (PSUM bank = 512 fp32)
N_TT = T // NT           # 4 t-tiles
N_TC = NT // 128         # 4 t-chunks per t-tile
N_EC = E // 128          # 8 e-chunks
N_SC = T // 128          # 16 s-chunks
VS = 132                 # v_aug free stride (129 used)
USE_FP8 = True
WSCALE = 32.0 if USE_FP8 else 1.0  # host pre-scale on Wq/Wk/bq/bk
SCALE = float(1.0 / (WSCALE * WSCALE * np.sqrt(D)))
N_WARM = 9

_cache: dict = {}


def _build(causal: bool):
    from contextlib import ExitStack
    import concourse.bass as bass
    import concourse.tile as tile
    from concourse import bacc, mybir
    from concourse.masks import make_identity

    f32 = mybir.dt.float32
    f16 = mybir.dt.float16
    f8 = mybir.dt.float8e4
    AF = mybir.ActivationFunctionType
    DR = mybir.MatmulPerfMode.DoubleRow

    nc = bacc.Bacc("TRN2", target_bir_lowering=False, debug=False,
                   num_devices=B)
    # packed inputs: [128, (tile, chunk, NT)] with x[t, e] at
    # [e % 128, tile(t), chunk(e), t % NT]
    xP16 = nc.dram_tensor("xP16", (128, N_TT * N_EC * NT), f16,
                          kind="ExternalInput").ap()
    Wq16 = nc.dram_tensor("Wq16", (128, N_EC * D), f16, kind="ExternalInput").ap()
    Wk16 = nc.dram_tensor("Wk16", (128, N_EC * D), f16, kind="ExternalInput").ap()
    Wv16 = nc.dram_tensor("Wv16", (128, N_EC * D), f16, kind="ExternalInput").ap()
    xP8 = Wq8 = Wk8 = None
    if USE_FP8:
        xP8 = nc.dram_tensor("xP8", (128, (N_TT - 1) * N_EC * NT), f8,
                             kind="ExternalInput").ap()
        Wq8 = nc.dram_tensor("Wq8", (128, N_EC * D), f8, kind="ExternalInput").ap()
        Wk8 = nc.dram_tensor("Wk8", (128, N_EC * D), f8, kind="ExternalInput").ap()
    bs = {p: nc.dram_tensor(f"b{p}", (D, 1), f32, kind="ExternalInput").ap()
          for p in "qkv"}
    out = nc.dram_tensor("out", (T, D), f16, kind="ExternalOutput").ap()

    with tile.TileContext(nc) as tc, ExitStack() as ctx:
        consts = ctx.enter_context(tc.tile_pool(name="consts", bufs=1))
        xt8_pool = ctx.enter_context(tc.tile_pool(name="xt8", bufs=3))
        xt16_pool = ctx.enter_context(tc.tile_pool(name="xt16", bufs=3))
        qT_pool = ctx.enter_context(tc.tile_pool(name="qT", bufs=2))
        vT_pool = ctx.enter_context(tc.tile_pool(name="vT", bufs=2))
        ex_pool = ctx.enter_context(tc.tile_pool(name="ex", bufs=3))
        small = ctx.enter_context(tc.tile_pool(name="small", bufs=8))
        outp = ctx.enter_context(tc.tile_pool(name="outp", bufs=8))
        ps_qkv = ctx.enter_context(tc.tile_pool(name="ps_qkv", bufs=2,
                                                space="PSUM"))
        ps_s = ctx.enter_context(tc.tile_pool(name="ps_s", bufs=3,
                                              space="PSUM"))
        ps_o = ctx.enter_context(tc.tile_pool(name="ps_o", bufs=2,
                                              space="PSUM"))
        ps_t = ctx.enter_context(tc.tile_pool(name="ps_t", bufs=1,
                                              space="PSUM"))

        # ---- DMAs: scalar ring carries weights, sync ring carries x ----
        # tile 0 computes q/k in fp16, so its critical path is Wq16+xt16_0.
        w16 = {p: consts.tile([128, N_EC, D], f16, tag=f"w16{p}",
                              name=f"w16{p}")
               for p in "qkv"}
        nc.scalar.dma_start(w16["q"][:].rearrange("p c d -> p (c d)"), Wq16[:])

        def load_x(pool, tag, dram, jj, dt, eng, pieces=1):
            # an x-tile as `pieces` separate tiles so dependencies unlock
            # per piece (tile deps are whole-tile, not subtile); each
            # piece is itself two chunk-range DMA transfers
            w = N_EC // pieces
            ph = []
            for h in range(pieces):
                xt = pool.tile([128, w, NT], dt, tag=f"{tag}{h}",
                               name=f"{tag}{h}")
                for g in range(2):
                    c0 = w * h + (w // 2) * g
                    t0 = (jj * N_EC + c0) * NT
                    eng.dma_start(
                        xt[:, (w // 2) * g:(w // 2) * (g + 1), :]
                        .rearrange("p c n -> p (c n)"),
                        dram[:, t0:t0 + (w // 2) * NT])
                ph.append((xt, w * h, w))
            return ph

        def xch(handle, c):
            # chunk accessor over a piece-list x handle
            for xt, c0, w in handle:
                if c0 <= c < c0 + w:
                    return xt[:, c - c0, :]
            raise KeyError(c)

        def xch2(handle, c):
            # two-chunk slice (DoubleRow) — never crosses a piece boundary
            for xt, c0, w in handle:
                if c0 <= 2 * c and 2 * c + 2 <= c0 + w:
                    return xt[:, 2 * c - c0:2 * c - c0 + 2, :]
            raise KeyError(c)

        def load_x16(jj):
            return load_x(xt16_pool, "xt16", xP16, jj, f16, nc.sync)

        def load_x8(jj):
            # fp8 x for t-tiles 1..3 (tile 0 is fp16-only); xP8 holds
            # tiles 1..3 back to back
            return load_x(xt8_pool, "xt8", xP8, jj - 1, f8, nc.sync)

        xt16_tiles = {0: load_x(xt16_pool, "xt16q", xP16, 0, f16, nc.sync,
                                pieces=4)}
        nc.scalar.dma_start(w16["k"][:].rearrange("p c d -> p (c d)"), Wk16[:])
        b_t = {}
        for p in "qkv":
            bt = consts.tile([128, 1], f32, tag=f"b{p}")
            nc.scalar.dma_start(bt[:], bs[p])
            b_t[p] = bt
        nc.scalar.dma_start(w16["v"][:].rearrange("p c d -> p (c d)"), Wv16[:])
        w8 = {}
        if USE_FP8:
            for p, W8 in (("q", Wq8), ("k", Wk8)):
                wt = consts.tile([128, N_EC, D], f8, tag=f"w8{p}")
                nc.scalar.dma_start(wt[:].rearrange("p c d -> p (c d)"), W8[:])
                w8[p] = wt
            xt8_tiles = {1: load_x8(1)} if N_TT > 1 else {}
        else:
            xt8_tiles = {}

        ident_h = consts.tile([128, 128], f16, tag="ident_h")
        make_identity(nc, ident_h[:])

        # PE warmup while the first x tile lands: HAM needs ~3.4us of
        # sustained PE activity to lift the clock gate to 8/8.  Routing
        # every warmup matmul through the single ps_t buffer serializes
        # them on the WAW hazard (~0.4us apiece), stretching a handful of
        # matmuls across the whole DMA wait.
        for _ in range(N_WARM):
            pw = ps_t.tile([128, 128], f32, tag="ps_t")
            nc.tensor.matmul(pw[:], ident_h[:], ident_h[:],
                             start=True, stop=True)

        masks_h = None
        if causal:
            # single lower-triangular (keep t>=s) 128x128 block
            masks_h = consts.tile([128, 128], f16, tag="masks_h")
            nc.gpsimd.memset(masks_h[:], 1.0)
            nc.gpsimd.affine_select(
                out=masks_h[:], in_=masks_h[:],
                compare_op=mybir.AluOpType.is_ge,
                fill=0.0, base=0, channel_multiplier=-1,
                pattern=[[1, 128]])

        kT_all = consts.tile([128, T], f16, tag="kT_all")
        v_all = consts.tile([128, N_SC, VS], f16, tag="v_all")
        nc.vector.memset(v_all[:, :, 128:129], 1.0)  # the ones columns

        qT_all = None
        if not causal:
            # full attention needs every t-tile's q resident before phase 2
            qT_all = consts.tile([128, T], f16, tag="qT_all")

        def proj(w16_t, w8_t, xt16, xt8, dest, bias, fp8):
            ps = ps_qkv.tile([128, NT], f32, tag="ps_qkv")
            if fp8:
                for c in range(N_EC // 2):
                    nc.tensor.matmul(
                        ps[:], w8_t[:, 2 * c:2 * c + 2, :], xch2(xt8, c),
                        start=(c == 0), stop=(c == N_EC // 2 - 1),
                        perf_mode=DR)
            else:
                for c in range(N_EC):
                    nc.tensor.matmul(
                        ps[:], w16_t[:, c, :], xch(xt16, c),
                        start=(c == 0), stop=(c == N_EC - 1))
            nc.vector.tensor_scalar_add(dest, ps[:], bias[:])

        def proj_qk(p, j, xt16, xt8, dest):
            fp8 = USE_FP8 and j > 0
            proj(w16[p], w8.get(p), xt16, xt8, dest, b_t[p], fp8)

        def proj_qk_fused16(xt16, qdest, kdest):
            # tile 0 only: q and k accumulate interleaved into two psum
            # banks so each arriving x piece feeds 4 back-to-back matmuls
            # (piece arrival paces the PE here; sequential q-then-k would
            # outrun the DMA on q and then replay k from sbuf cold)
            psq = ps_qkv.tile([128, NT], f32, tag="ps_qkv")
            psk = ps_qkv.tile([128, NT], f32, tag="ps_qkv")
            for c in range(N_EC):
                xc = xch(xt16, c)
                nc.tensor.matmul(psq[:], w16["q"][:, c, :], xc,
                                 start=(c == 0), stop=(c == N_EC - 1),
                                 skip_group_check=True)
                nc.tensor.matmul(psk[:], w16["k"][:, c, :], xc,
                                 start=(c == 0), stop=(c == N_EC - 1),
                                 skip_group_check=True)
            nc.vector.tensor_scalar_add(qdest, psq[:], b_t["q"][:])
            nc.vector.tensor_scalar_add(kdest, psk[:], b_t["k"][:])

        def scores_exp(j, qT, ex_all, i0, i1):
            # Diagonal s-chunk m: columns t_local < 128*m are never read by
            # PV (those t-chunks exclude this s-chunk), so compute only
            # [128*m:NT] and mask just the 128-wide diagonal sub-block.
            for i in range(i0, i1):
                m = i - j * N_TC
                off = 128 * m if (causal and m > 0) else 0
                ps = ps_s.tile([128, NT], f32, tag="ps_s")
                nc.tensor.matmul(ps[:, off:NT],
                                 kT_all[:, i * 128:(i + 1) * 128],
                                 qT[:, off:NT], start=True, stop=True)
                ex = ex_all[:, i * NT + off:(i + 1) * NT]
                nc.scalar.activation(ex, ps[:, off:NT], AF.Exp, scale=SCALE)
                if causal and m >= 0:
                    # gpsimd is otherwise idle; keeping the diagonal mask
                    # off the DVE shortens the exp->PV dependency chain
                    nc.gpsimd.tensor_mul(
                        ex_all[:, i * NT + off:i * NT + off + 128],
                        ex_all[:, i * NT + off:i * NT + off + 128],
                        masks_h[:])

        def v_proj_transpose(j, xt16):
            vT = vT_pool.tile([128, NT], f16, tag="vT")
            proj(w16["v"], None, xt16, None, vT[:], b_t["v"], False)
            for tch in range(N_TC):
                sc = j * N_TC + tch
                pt = ps_t.tile([128, 256], f16, tag="ps_t")
                nc.tensor.transpose(pt[:, 0:128],
                                    vT[:, tch * 128:(tch + 1) * 128],
                                    ident_h[:])
                nc.vector.tensor_copy(v_all[:, sc, 0:128], pt[:, 0:128])

        def pv_chunk(j, ex_all, tch):
            # PV natural for one t-chunk; denominator rides in column 128
            tc_glob = j * N_TC + tch
            n_i = tc_glob + 1 if causal else N_SC
            po = ps_o.tile([128, VS], f32, tag="ps_o")
            for i in range(n_i):
                nc.tensor.matmul(
                    po[:, 0:129],
                    ex_all[:, i * NT + tch * 128:i * NT + (tch + 1) * 128],
                    v_all[:, i, 0:129],
                    start=(i == 0), stop=(i == n_i - 1),
                    skip_group_check=True)
            rec = small.tile([128, 1], f32, tag="rec")
            nc.vector.reciprocal(rec[:], po[:, 128:129])
            ot = outp.tile([128, 128], f16, tag="ot")
            nc.vector.tensor_scalar_mul(ot[:], po[:, 0:128], rec[:])
            r0 = tc_glob * 128
            nc.sync.dma_start(out[r0:r0 + 128, :], ot[:])

        def pv_out(j, ex_all):
            for tch in range(N_TC):
                pv_chunk(j, ex_all, tch)

        if causal:
            prev = None
            for j in range(N_TT):
                t0 = j * NT
                last = j == N_TT - 1
                xt16 = xt16_tiles.pop(j)
                xt8 = xt8_tiles.pop(j, None)
                qT = qT_pool.tile([128, NT], f16, tag="qT")
                if USE_FP8 and j == 0:
                    proj_qk_fused16(xt16, qT[:], kT_all[:, t0:t0 + NT])
                else:
                    proj_qk("q", j, xt16, xt8, qT[:])
                    proj_qk("k", j, xt16, xt8, kT_all[:, t0:t0 + NT])
                if j + 1 < N_TT:
                    xt16_tiles[j + 1] = load_x16(j + 1)
                    if USE_FP8 and j + 2 < N_TT:
                        xt8_tiles[j + 2] = load_x8(j + 2)
                ex_all = ex_pool.tile([128, N_SC * NT], f16, tag="ex")
                n_sc = (j + 1) * N_TC
                if not last:
                    scores_exp(j, qT, ex_all, 0, n_sc)
                    v_proj_transpose(j, xt16)
                    # PV runs one tile behind: the in-order PE then fills
                    # this tile's exp-chain wait with the next tile's
                    # projections instead of stalling on PV's last-chunk
                    # dependency.
                    if prev is not None:
                        pv_out(*prev)
                    prev = (j, ex_all)
                else:
                    # last tile: no next-tile work exists to hide the PV
                    # tail, so interleave its PV chunks into the scores
                    # stream as their diagonal ex chunks become ready.
                    v_proj_transpose(j, xt16)
                    scores_exp(j, qT, ex_all, 0, j * N_TC + 2)
                    if prev is not None:
                        pv_out(*prev)
                    for tch in range(N_TC):
                        if j * N_TC + tch + 2 <= n_sc - 1:
                            scores_exp(j, qT, ex_all, j * N_TC + tch + 2,
                                       j * N_TC + tch + 3)
                        pv_chunk(j, ex_all, tch)
        else:
            # phase 1: all projections; phase 2: attention per t-tile
            for j in range(N_TT):
                t0 = j * NT
                xt16 = xt16_tiles.pop(j)
                xt8 = xt8_tiles.pop(j, None)
                proj_qk("q", j, xt16, xt8, qT_all[:, t0:t0 + NT])
                proj_qk("k", j, xt16, xt8, kT_all[:, t0:t0 + NT])
                v_proj_transpose(j, xt16)
                if j + 1 < N_TT:
                    xt16_tiles[j + 1] = load_x16(j + 1)
                    if USE_FP8 and j + 2 < N_TT:
                        xt8_tiles[j + 2] = load_x8(j + 2)
            for j in range(N_TT):
                ex_all = ex_pool.tile([128, N_SC * NT], f16, tag="ex")
                scores_exp(j, qT_all[:, j * NT:(j + 1) * NT], ex_all,
                           0, N_SC)
                pv_out(j, ex_all)

    nc.compile()
    return nc


def _get(causal: bool):
    if causal not in _cache:
        _cache[causal] = _build(causal)
    return _cache[causal]


def _pack_x(xb, np_dtype):
    # [T, E] -> [128, (tile, chunk, NT)]: x[t, e] at
    # [e % 128, tile(t), chunk(e), t % NT]
    p = xb.reshape(-1, NT, N_EC, 128).transpose(3, 0, 2, 1)
    return np.ascontiguousarray(p.astype(np_dtype).reshape(128, -1))


def _pack_w(w, np_dtype):
    # [E, D] -> [128, (chunk, D)]: W[e, d] at [e % 128, chunk(e), d]
    p = w.reshape(N_EC, 128, D).transpose(1, 0, 2)
    return np.ascontiguousarray(p.astype(np_dtype).reshape(128, -1))


def _make_in_maps(x, Wq, bq, Wk, bk, Wv, bv):
    x = np.asarray(x, dtype=np.float32)
    Wq_s = np.asarray(Wq, np.float32) * WSCALE
    Wk_s = np.asarray(Wk, np.float32) * WSCALE
    shared = {
        "Wq16": _pack_w(Wq_s, np.float16),
        "Wk16": _pack_w(Wk_s, np.float16),
        "Wv16": _pack_w(np.asarray(Wv, np.float32), np.float16),
        "bq": np.ascontiguousarray(
            np.asarray(bq, np.float32).reshape(D, 1) * WSCALE),
        "bk": np.ascontiguousarray(
            np.asarray(bk, np.float32).reshape(D, 1) * WSCALE),
        "bv": np.ascontiguousarray(np.asarray(bv, np.float32).reshape(D, 1)),
    }
    f8 = None
    if USE_FP8:
        import ml_dtypes
        f8 = ml_dtypes.float8_e4m3
        shared["Wq8"] = _pack_w(Wq_s, f8)
        shared["Wk8"] = _pack_w(Wk_s, f8)
    in_maps = []
    for b in range(B):
        m = dict(shared)
        m["xP16"] = _pack_x(x[b], np.float16)
        if USE_FP8:
            m["xP8"] = _pack_x(x[b][NT:], f8)
        in_maps.append(m)
    return in_maps


def kernel(x, Wq, bq, Wk, bk, Wv, bv, mask, **_ignored):
    from concourse.bass_utils import run_bass_kernel_spmd

    causal = bool(np.asarray(mask).item()) if mask is not None else False
    nc = _get(causal)
    in_maps = _make_in_maps(x, Wq, bq, Wk, bk, Wv, bv)
    res = run_bass_kernel_spmd(nc, in_maps, core_ids=list(range(B)))
    return np.stack([res.results[b]["out"] for b in range(B)],
                    axis=0).astype(np.float32)
